# revision 12
# baseline (speedup 1.0000x reference)
"""Trainium2 Bass kernel for nn_BigramLanguageModel (dense transformer block).

Reference computation (B=2, T=2048, E=1024, V=32000):
    x      = emb_table[X] + pos_table                       # [B,T,E]
    k,q,v  = x@Wk, x@Wq, x@Wv                               # [B,T,E]
    w      = (q @ k^T) / sqrt(E), causal mask (tril)        # [B,T,T]
    w      = softmax(w, axis=1)          # QUIRK: over the *query* axis
    out    = w @ v                                          # [B,T,E]
    logits = out @ Wro + bro                                # [B,T,V]

Sharding: 8 cores = 2 (batch) x 4 (vocab slices of 8000 for the readout
matmul, which dominates FLOPs).  In "cc" mode the attention is further
sharded over key tiles inside each batch group (interleaved: the core
with vocab index dv owns global k-tiles {dv + 4j}), with an AllReduce of
partial attention outputs; the interleaving keeps the SPMD program
identical across cores.  In "rep" mode each core computes the full
attention for its batch (no collective).

Device-side layout: scores are computed transposed, wT[k,q], so the
softmax-over-q runs along the free axis.  The softmax denominator depends
only on k, so it is folded into V (V' = V/denom[k]) and the attention
output is produced directly in outT[e,q] layout — exactly the lhsT layout
the readout matmul wants.  Causal masking uses the block structure: chunks
with q_end <= k0 are never computed nor read; only the single diagonal
512-chunk per k-tile needs an additive staircase mask.

All matmul operands are bf16 (full PE rate), accumulation fp32 in PSUM.
"""

import sys

if "/opt/trn_rl_repo" not in sys.path:
    sys.path.insert(0, "/opt/trn_rl_repo")

from contextlib import ExitStack

import numpy as np
import ml_dtypes

import concourse.bass as bass
import concourse.tile as tile
from concourse import bacc, mybir
from concourse.bass_utils import run_bass_kernel_spmd

P = 128
B, T, E, VOC = 2, 2048, 1024, 32000
VSPLIT = 4                # vocab splits per batch group
VS = VOC // VSPLIT        # 8000 vocab columns per core
NE = E // P               # 8 embedding partition-tiles
NT = T // P               # 16 token partition-tiles
KL = NT // VSPLIT         # 4 local k-tiles per core (interleaved by dv)
TK = KL * P               # 512 key tokens per core
QCH = 512                 # q chunk width
NQC = T // QCH            # 4
VCH = 500                 # vocab chunk width (<=512 psum bank, 8000 = 16*500)
NVC = VS // VCH           # 16
SCALE = 1.0 / 32.0        # 1/sqrt(E)
MASK_VAL = -960000.0      # additive pre-scale mask; /32 -> -30000 -> exp = 0

BF16 = mybir.dt.bfloat16
F32 = mybir.dt.float32

# "rep3" (default): rep2 + fp8-DoubleRow k/q projections and score matmuls
#   + bf16 logits output (halves the dominant output-write DMA).
# "rep2": replicated attention per batch group, weight-amortized
#   loop order, all-bf16 (~338 us/core on HW).
# "rep": replicated attention, naive loop order (~476 us).
# "cc": k-sharded attention + AllReduce (~520 us; collective costs more
#   than the compute it saves at this size).
MODE = "rep3"

F8 = mybir.dt.float8e4
DR = mybir.MatmulPerfMode.DoubleRow
XS = 128.0       # x fp8 storage scale
WS = 256.0       # Wq/Wk fp8 storage scale
QS = 64.0        # q/k fp8 storage scale
S_PROJ = QS / (XS * WS)      # PSUM -> fp8 q/k copy scale
S_EXP = SCALE / (QS * QS)    # exp activation scale on fp8-scored PSUM
MASK_R3 = -1.0e9             # additive mask on raw scores (pre-activation)
OS = 128.0                   # attention-output fp8 storage scale (rep4)
ROW_S = 256.0                # Wro fp8 storage scale (rep4)
S_RO = 1.0 / (OS * ROW_S)    # readout PSUM -> bf16 logits scale (rep4)

_CACHE: dict = {}


def _emit_body_cc(tc, nc, aps, sfx):
    """k-sharded attention (interleaved) + AllReduce + vocab-sliced readout."""
    xT_d, xTk_d, wk_d, wq_d, wv_d, wro_d, mask_d, out_d = aps
    Exp = mybir.ActivationFunctionType.Exp
    groups = [[0, 1, 2, 3], [4, 5, 6, 7]]

    with ExitStack() as root:
        misc = root.enter_context(tc.tile_pool(name=f"misc{sfx}", bufs=1))
        psum = root.enter_context(tc.tile_pool(name=f"psum{sfx}", bufs=6, space="PSUM"))
        stage = root.enter_context(tc.tile_pool(name=f"stage{sfx}", bufs=2))
        dram = root.enter_context(tc.tile_pool(name=f"dram{sfx}", bufs=1, space="DRAM"))

        mask_t = misc.tile([P, QCH], F32, tag="mask", name=f"mask_t{sfx}")
        nc.sync.dma_start(mask_t[:], mask_d[:])
        parts_t = misc.tile([P, KL, NQC], F32, tag="parts", name=f"parts_t{sfx}")
        denom_t = misc.tile([P, KL], F32, tag="denom", name=f"denom_t{sfx}")
        recip_t = misc.tile([P, KL], F32, tag="recip", name=f"recip_t{sfx}")

        cc_in = dram.tile([NE, P, T], BF16, tag="cci", name=f"cc_in{sfx}")
        cc_out = dram.tile([NE, P, T], BF16, tag="cco", name=f"cc_out{sfx}")

        pkv = root.enter_context(tc.tile_pool(name=f"pkv{sfx}", bufs=1))
        kTl_t = [pkv.tile([P, TK], BF16, tag=f"kTl{i}", name=f"kTl{i}{sfx}") for i in range(NE)]
        vl_t = [pkv.tile([P, E], BF16, tag=f"vl{i}", name=f"vl{i}{sfx}") for i in range(KL)]

        pq = ExitStack()
        q_pool = pq.enter_context(tc.tile_pool(name=f"pq{sfx}", bufs=1))
        qT_t = [q_pool.tile([P, T], BF16, tag=f"qT{i}", name=f"qT{i}{sfx}") for i in range(NE)]

        p1 = ExitStack()
        x_pool = p1.enter_context(tc.tile_pool(name=f"px{sfx}", bufs=1))
        w_pool = p1.enter_context(tc.tile_pool(name=f"pw{sfx}", bufs=2))

        # ================= phase 1: projections =====================
        xT_t = [x_pool.tile([P, T], BF16, tag=f"xT{i}", name=f"xT{i}{sfx}") for i in range(NE)]
        for e in range(NE):
            nc.sync.dma_start(xT_t[e][:], xT_d[e])
        xTk_t = [x_pool.tile([P, TK], BF16, tag=f"xTk{i}", name=f"xTk{i}{sfx}") for i in range(NE)]
        for e in range(NE):
            nc.sync.dma_start(xTk_t[e][:], xTk_d[e])

        wq_t = w_pool.tile([P, NE, E], BF16, tag="w", name=f"wq_t{sfx}")
        for e in range(NE):
            nc.sync.dma_start(wq_t[:, e, :], wq_d[e])
        wk_t = w_pool.tile([P, NE, E], BF16, tag="w", name=f"wk_t{sfx}")
        for e in range(NE):
            nc.sync.dma_start(wk_t[:, e, :], wk_d[e])

        # qT[eo, t] = sum_e Wq[e, eo] * xT[e, t]   (full, replicated)
        for eo in range(NE):
            for tch in range(NQC):
                ps = psum.tile([P, QCH], F32, tag="mm", name=f"ps_q{eo}_{tch}{sfx}")
                for e in range(NE):
                    nc.tensor.matmul(
                        ps[:],
                        wq_t[:, e, eo * P:(eo + 1) * P],
                        xT_t[e][:, tch * QCH:(tch + 1) * QCH],
                        start=(e == 0), stop=(e == NE - 1),
                    )
                nc.scalar.copy(qT_t[eo][:, tch * QCH:(tch + 1) * QCH], ps[:])

        wv_t = w_pool.tile([P, NE, E], BF16, tag="w", name=f"wv_t{sfx}")
        for e in range(NE):
            nc.sync.dma_start(wv_t[:, e, :], wv_d[e])

        # kTl[eo, kloc] = sum_e Wk[e, eo] * xTk[e, kloc]  (local k slice)
        for eo in range(NE):
            ps = psum.tile([P, TK], F32, tag="mm", name=f"ps_k{eo}{sfx}")
            for e in range(NE):
                nc.tensor.matmul(
                    ps[:],
                    wk_t[:, e, eo * P:(eo + 1) * P],
                    xTk_t[e][:],
                    start=(e == 0), stop=(e == NE - 1),
                )
            nc.scalar.copy(kTl_t[eo][:], ps[:])

        # Vl[kloc, e] = sum_e' xTk[e', kloc]^T * Wv[e', e]
        for tb in range(KL):
            for ec in range(E // QCH):
                ps = psum.tile([P, QCH], F32, tag="mm", name=f"ps_v{tb}_{ec}{sfx}")
                for e in range(NE):
                    nc.tensor.matmul(
                        ps[:],
                        xTk_t[e][:, tb * P:(tb + 1) * P],
                        wv_t[:, e, ec * QCH:(ec + 1) * QCH],
                        start=(e == 0), stop=(e == NE - 1),
                    )
                nc.scalar.copy(vl_t[tb][:, ec * QCH:(ec + 1) * QCH], ps[:])

        p1.close()  # frees xT/xTk + W pools

        # ============ phase 2: scores + softmax-over-q ==============
        # local tile j == global k-tile dv+4j; diagonal chunk qc = j
        p2 = ExitStack()
        wT_pool = p2.enter_context(tc.tile_pool(name=f"pwT{sfx}", bufs=1, side="right"))
        wTl_t = [wT_pool.tile([P, T], BF16, tag=f"wTl{i}", name=f"wTl{i}{sfx}") for i in range(KL)]

        for j in range(KL):
            for qc in range(j, NQC):
                ps = psum.tile([P, QCH], F32, tag="mm", name=f"ps_s{j}_{qc}{sfx}")
                for e in range(NE):
                    nc.tensor.matmul(
                        ps[:],
                        kTl_t[e][:, j * P:(j + 1) * P],
                        qT_t[e][:, qc * QCH:(qc + 1) * QCH],
                        start=(e == 0), stop=(e == NE - 1),
                    )
                wslice = wTl_t[j][:, qc * QCH:(qc + 1) * QCH]
                acc = parts_t[:, j, qc:qc + 1]
                if qc == j:
                    stg = stage.tile([P, QCH], F32, tag="stg", name=f"stg{j}{sfx}")
                    nc.vector.tensor_add(stg[:], ps[:], mask_t[:])
                    nc.scalar.activation(wslice, stg[:], Exp, bias=0.0,
                                         scale=SCALE, accum_out=acc)
                else:
                    nc.scalar.activation(wslice, ps[:], Exp, bias=0.0,
                                         scale=SCALE, accum_out=acc)
            nc.vector.reduce_sum(denom_t[:, j:j + 1], parts_t[:, j, j:NQC],
                                 axis=mybir.AxisListType.X)
            nc.vector.reciprocal(recip_t[:, j:j + 1], denom_t[:, j:j + 1])
            nc.vector.tensor_scalar_mul(vl_t[j][:], vl_t[j][:], recip_t[:, j:j + 1])

        pq.close()  # frees qT

        # ====== phase 3: partial outT[e,q] -> AllReduce =============
        po_stage = ExitStack()
        av_pool = po_stage.enter_context(tc.tile_pool(name=f"pav{sfx}", bufs=4))
        for eb in range(NE):
            for qc in range(NQC):
                ps = psum.tile([P, QCH], F32, tag="mm", name=f"ps_o{eb}_{qc}{sfx}")
                njs = qc + 1  # local tiles j <= qc contribute
                for j in range(njs):
                    nc.tensor.matmul(
                        ps[:],
                        vl_t[j][:, eb * P:(eb + 1) * P],
                        wTl_t[j][:, qc * QCH:(qc + 1) * QCH],
                        start=(j == 0), stop=(j == njs - 1),
                    )
                stg = av_pool.tile([P, QCH], BF16, tag="av", name=f"av{eb}_{qc}{sfx}")
                nc.scalar.copy(stg[:], ps[:])
                nc.sync.dma_start(cc_in[eb, :, qc * QCH:(qc + 1) * QCH], stg[:])
        po_stage.close()
        p2.close()  # frees wTl

        nc.gpsimd.collective_compute(
            "AllReduce", mybir.AluOpType.add,
            replica_groups=groups,
            ins=[cc_in.opt()], outs=[cc_out.opt()],
        )

        # ============ phase 4: readout ==============================
        p3 = ExitStack()
        outT_pool = p3.enter_context(tc.tile_pool(name=f"poutT{sfx}", bufs=1))
        outT_t = [outT_pool.tile([P, T], BF16, tag=f"oT{i}", name=f"oT{i}{sfx}") for i in range(NE)]
        for e in range(NE):
            nc.sync.dma_start(outT_t[e][:], cc_out[e])

        p4 = ExitStack()
        ro_pool = p4.enter_context(tc.tile_pool(name=f"pro{sfx}", bufs=2))
        ostg_pool = p4.enter_context(tc.tile_pool(name=f"postg{sfx}", bufs=4))

        for vc in range(NVC):
            wro_t = ro_pool.tile([P, NE, VCH], BF16, tag="wro", name=f"wro{vc}{sfx}")
            for e in range(NE):
                nc.sync.dma_start(wro_t[:, e, :], wro_d[e, :, vc * VCH:(vc + 1) * VCH])
            for tb in range(NT):
                ps = psum.tile([P, VCH], F32, tag="mm", name=f"ps_r{vc}_{tb}{sfx}")
                for e in range(NE):
                    nc.tensor.matmul(
                        ps[:],
                        outT_t[e][:, tb * P:(tb + 1) * P],
                        wro_t[:, e, :],
                        start=(e == 0), stop=(e == NE - 1),
                    )
                stg = ostg_pool.tile([P, VCH], F32, tag="ostg", name=f"ostg{vc}_{tb}{sfx}")
                if tb % 2 == 0:
                    nc.vector.tensor_copy(stg[:], ps[:])
                else:
                    nc.scalar.copy(stg[:], ps[:])
                nc.sync.dma_start(out_d[tb, :, vc * VCH:(vc + 1) * VCH], stg[:])

        p4.close()
        p3.close()


def _emit_body_rep(tc, nc, aps, sfx):
    """Replicated attention per core (no collective) + vocab-sliced readout."""
    xT_d, xTk_d, wk_d, wq_d, wv_d, wro_d, mask4_d, out_d = aps
    Exp = mybir.ActivationFunctionType.Exp

    with ExitStack() as root:
        misc = root.enter_context(tc.tile_pool(name=f"misc{sfx}", bufs=1))
        psum = root.enter_context(tc.tile_pool(name=f"psum{sfx}", bufs=6, space="PSUM"))
        stage = root.enter_context(tc.tile_pool(name=f"stage{sfx}", bufs=2))

        mask_t = misc.tile([P, NQC, QCH], F32, tag="mask", name=f"mask_t{sfx}")
        nc.sync.dma_start(mask_t[:], mask4_d[:])
        parts_t = misc.tile([P, NT, NQC], F32, tag="parts", name=f"parts_t{sfx}")
        denom_t = misc.tile([P, NT], F32, tag="denom", name=f"denom_t{sfx}")
        recip_t = misc.tile([P, NT], F32, tag="recip", name=f"recip_t{sfx}")

        pv = root.enter_context(tc.tile_pool(name=f"pv{sfx}", bufs=1))
        v_t = [pv.tile([P, E], BF16, tag=f"v{i}", name=f"v{i}{sfx}") for i in range(NT)]

        pkq = ExitStack()
        kq_pool = pkq.enter_context(tc.tile_pool(name=f"pkq{sfx}", bufs=1))
        kT_t = [kq_pool.tile([P, T], BF16, tag=f"kT{i}", name=f"kT{i}{sfx}") for i in range(NE)]
        qT_t = [kq_pool.tile([P, T], BF16, tag=f"qT{i}", name=f"qT{i}{sfx}") for i in range(NE)]

        p1 = ExitStack()
        x_pool = p1.enter_context(tc.tile_pool(name=f"px{sfx}", bufs=1))
        w_pool = p1.enter_context(tc.tile_pool(name=f"pw{sfx}", bufs=2))

        xT_t = [x_pool.tile([P, T], BF16, tag=f"xT{i}", name=f"xT{i}{sfx}") for i in range(NE)]
        for e in range(NE):
            nc.sync.dma_start(xT_t[e][:], xT_d[e])

        wk_t = w_pool.tile([P, NE, E], BF16, tag="w", name=f"wk_t{sfx}")
        for e in range(NE):
            nc.sync.dma_start(wk_t[:, e, :], wk_d[e])
        wq_t = w_pool.tile([P, NE, E], BF16, tag="w", name=f"wq_t{sfx}")
        for e in range(NE):
            nc.sync.dma_start(wq_t[:, e, :], wq_d[e])

        for eo in range(NE):
            for tch in range(NQC):
                ps = psum.tile([P, QCH], F32, tag="mm", name=f"ps_k{eo}_{tch}{sfx}")
                for e in range(NE):
                    nc.tensor.matmul(
                        ps[:],
                        wk_t[:, e, eo * P:(eo + 1) * P],
                        xT_t[e][:, tch * QCH:(tch + 1) * QCH],
                        start=(e == 0), stop=(e == NE - 1),
                    )
                nc.scalar.copy(kT_t[eo][:, tch * QCH:(tch + 1) * QCH], ps[:])

        wv_t = w_pool.tile([P, NE, E], BF16, tag="w", name=f"wv_t{sfx}")
        for e in range(NE):
            nc.sync.dma_start(wv_t[:, e, :], wv_d[e])

        for eo in range(NE):
            for tch in range(NQC):
                ps = psum.tile([P, QCH], F32, tag="mm", name=f"ps_q{eo}_{tch}{sfx}")
                for e in range(NE):
                    nc.tensor.matmul(
                        ps[:],
                        wq_t[:, e, eo * P:(eo + 1) * P],
                        xT_t[e][:, tch * QCH:(tch + 1) * QCH],
                        start=(e == 0), stop=(e == NE - 1),
                    )
                nc.scalar.copy(qT_t[eo][:, tch * QCH:(tch + 1) * QCH], ps[:])

        for tb in range(NT):
            for ec in range(E // QCH):
                ps = psum.tile([P, QCH], F32, tag="mm", name=f"ps_v{tb}_{ec}{sfx}")
                for e in range(NE):
                    nc.tensor.matmul(
                        ps[:],
                        xT_t[e][:, tb * P:(tb + 1) * P],
                        wv_t[:, e, ec * QCH:(ec + 1) * QCH],
                        start=(e == 0), stop=(e == NE - 1),
                    )
                nc.scalar.copy(v_t[tb][:, ec * QCH:(ec + 1) * QCH], ps[:])

        p1.close()

        p2 = ExitStack()
        wT_pool = p2.enter_context(tc.tile_pool(name=f"pwT{sfx}", bufs=1, side="right"))
        wT_t = [wT_pool.tile([P, T], BF16, tag=f"wT{i}", name=f"wT{i}{sfx}") for i in range(NT)]

        for kt in range(NT):
            qcd = kt // 4
            for qc in range(qcd, NQC):
                ps = psum.tile([P, QCH], F32, tag="mm", name=f"ps_s{kt}_{qc}{sfx}")
                for e in range(NE):
                    nc.tensor.matmul(
                        ps[:],
                        kT_t[e][:, kt * P:(kt + 1) * P],
                        qT_t[e][:, qc * QCH:(qc + 1) * QCH],
                        start=(e == 0), stop=(e == NE - 1),
                    )
                wslice = wT_t[kt][:, qc * QCH:(qc + 1) * QCH]
                acc = parts_t[:, kt, qc:qc + 1]
                if qc == qcd:
                    dv = kt % 4
                    stg = stage.tile([P, QCH], F32, tag="stg", name=f"stg{kt}{sfx}")
                    nc.vector.tensor_add(stg[:], ps[:], mask_t[:, dv, :])
                    nc.scalar.activation(wslice, stg[:], Exp, bias=0.0,
                                         scale=SCALE, accum_out=acc)
                else:
                    nc.scalar.activation(wslice, ps[:], Exp, bias=0.0,
                                         scale=SCALE, accum_out=acc)
            nc.vector.reduce_sum(denom_t[:, kt:kt + 1], parts_t[:, kt, qcd:NQC],
                                 axis=mybir.AxisListType.X)
            nc.vector.reciprocal(recip_t[:, kt:kt + 1], denom_t[:, kt:kt + 1])
            nc.vector.tensor_scalar_mul(v_t[kt][:], v_t[kt][:], recip_t[:, kt:kt + 1])

        pkq.close()

        p3 = ExitStack()
        outT_pool = p3.enter_context(tc.tile_pool(name=f"poutT{sfx}", bufs=1))
        outT_t = [outT_pool.tile([P, T], BF16, tag=f"oT{i}", name=f"oT{i}{sfx}") for i in range(NE)]

        for eb in range(NE):
            for qc in range(NQC):
                kts = list(range(0, (qc + 1) * 4))
                ps = psum.tile([P, QCH], F32, tag="mm", name=f"ps_o{eb}_{qc}{sfx}")
                for i, kt in enumerate(kts):
                    nc.tensor.matmul(
                        ps[:],
                        v_t[kt][:, eb * P:(eb + 1) * P],
                        wT_t[kt][:, qc * QCH:(qc + 1) * QCH],
                        start=(i == 0), stop=(i == len(kts) - 1),
                    )
                nc.scalar.copy(outT_t[eb][:, qc * QCH:(qc + 1) * QCH], ps[:])

        p2.close()

        p4 = ExitStack()
        ro_pool = p4.enter_context(tc.tile_pool(name=f"pro{sfx}", bufs=2))
        ostg_pool = p4.enter_context(tc.tile_pool(name=f"postg{sfx}", bufs=4))

        for vc in range(NVC):
            wro_t = ro_pool.tile([P, NE, VCH], BF16, tag="wro", name=f"wro{vc}{sfx}")
            for e in range(NE):
                nc.sync.dma_start(wro_t[:, e, :], wro_d[e, :, vc * VCH:(vc + 1) * VCH])
            for tb in range(NT):
                ps = psum.tile([P, VCH], F32, tag="mm", name=f"ps_r{vc}_{tb}{sfx}")
                for e in range(NE):
                    nc.tensor.matmul(
                        ps[:],
                        outT_t[e][:, tb * P:(tb + 1) * P],
                        wro_t[:, e, :],
                        start=(e == 0), stop=(e == NE - 1),
                    )
                stg = ostg_pool.tile([P, VCH], F32, tag="ostg", name=f"ostg{vc}_{tb}{sfx}")
                if tb % 2 == 0:
                    nc.vector.tensor_copy(stg[:], ps[:])
                else:
                    nc.scalar.copy(stg[:], ps[:])
                nc.sync.dma_start(out_d[tb, :, vc * VCH:(vc + 1) * VCH], stg[:])

        p4.close()
        p3.close()


def _emit_body_rep2(tc, nc, aps, sfx):
    """Replicated attention, weight-amortized loop order: each stationary
    operand (lhsT) is reused across several matmuls feeding parallel PSUM
    banks, so LDWEIGHTS traffic drops ~3-4x."""
    xT_d, xTk_d, wk_d, wq_d, wv_d, wro_d, mask4_d, out_d = aps
    Exp = mybir.ActivationFunctionType.Exp

    with ExitStack() as root:
        misc = root.enter_context(tc.tile_pool(name=f"misc{sfx}", bufs=1))
        psum = root.enter_context(tc.tile_pool(name=f"psum{sfx}", bufs=8, space="PSUM"))
        stage = root.enter_context(tc.tile_pool(name=f"stage{sfx}", bufs=2))

        mask_t = misc.tile([P, NQC, QCH], F32, tag="mask", name=f"mask_t{sfx}")
        nc.sync.dma_start(mask_t[:], mask4_d[:])
        parts_t = misc.tile([P, NT, NQC], F32, tag="parts", name=f"parts_t{sfx}")
        denom_t = misc.tile([P, NT], F32, tag="denom", name=f"denom_t{sfx}")
        recip_t = misc.tile([P, NT], F32, tag="recip", name=f"recip_t{sfx}")

        pv = root.enter_context(tc.tile_pool(name=f"pv{sfx}", bufs=1))
        v_t = [pv.tile([P, E], BF16, tag=f"v{i}", name=f"v{i}{sfx}") for i in range(NT)]

        pkq = ExitStack()
        kq_pool = pkq.enter_context(tc.tile_pool(name=f"pkq{sfx}", bufs=1))
        kT_t = [kq_pool.tile([P, T], BF16, tag=f"kT{i}", name=f"kT{i}{sfx}") for i in range(NE)]
        qT_t = [kq_pool.tile([P, T], BF16, tag=f"qT{i}", name=f"qT{i}{sfx}") for i in range(NE)]

        p1 = ExitStack()
        x_pool = p1.enter_context(tc.tile_pool(name=f"px{sfx}", bufs=1))
        w_pool = p1.enter_context(tc.tile_pool(name=f"pw{sfx}", bufs=2))

        xT_t = [x_pool.tile([P, T], BF16, tag=f"xT{i}", name=f"xT{i}{sfx}") for i in range(NE)]
        for e in range(NE):
            nc.sync.dma_start(xT_t[e][:], xT_d[e])

        wk_t = w_pool.tile([P, NE, E], BF16, tag="w", name=f"wk_t{sfx}")
        for e in range(NE):
            nc.sync.dma_start(wk_t[:, e, :], wk_d[e])
        wq_t = w_pool.tile([P, NE, E], BF16, tag="w", name=f"wq_t{sfx}")
        for e in range(NE):
            nc.sync.dma_start(wq_t[:, e, :], wq_d[e])

        # kT: weight wk[:,e,eo] loaded once per (eo,e), reused for 4 t-chunks
        for eo in range(NE):
            psA = [psum.tile([P, QCH], F32, tag="mm", name=f"ps_k{eo}_{t_}{sfx}")
                   for t_ in range(NQC)]
            for e in range(NE):
                for tch in range(NQC):
                    nc.tensor.matmul(
                        psA[tch][:],
                        wk_t[:, e, eo * P:(eo + 1) * P],
                        xT_t[e][:, tch * QCH:(tch + 1) * QCH],
                        start=(e == 0), stop=(e == NE - 1),
                    )
            for tch in range(NQC):
                nc.scalar.copy(kT_t[eo][:, tch * QCH:(tch + 1) * QCH], psA[tch][:])

        wv_t = w_pool.tile([P, NE, E], BF16, tag="w", name=f"wv_t{sfx}")
        for e in range(NE):
            nc.sync.dma_start(wv_t[:, e, :], wv_d[e])

        for eo in range(NE):
            psA = [psum.tile([P, QCH], F32, tag="mm", name=f"ps_q{eo}_{t_}{sfx}")
                   for t_ in range(NQC)]
            for e in range(NE):
                for tch in range(NQC):
                    nc.tensor.matmul(
                        psA[tch][:],
                        wq_t[:, e, eo * P:(eo + 1) * P],
                        xT_t[e][:, tch * QCH:(tch + 1) * QCH],
                        start=(e == 0), stop=(e == NE - 1),
                    )
            for tch in range(NQC):
                nc.scalar.copy(qT_t[eo][:, tch * QCH:(tch + 1) * QCH], psA[tch][:])

        # V: weight xT[e][:,tb] loaded once per (tb,e), reused for 2 e-chunks
        for tb in range(NT):
            psA = [psum.tile([P, QCH], F32, tag="mm", name=f"ps_v{tb}_{ec}{sfx}")
                   for ec in range(E // QCH)]
            for e in range(NE):
                for ec in range(E // QCH):
                    nc.tensor.matmul(
                        psA[ec][:],
                        xT_t[e][:, tb * P:(tb + 1) * P],
                        wv_t[:, e, ec * QCH:(ec + 1) * QCH],
                        start=(e == 0), stop=(e == NE - 1),
                    )
            for ec in range(E // QCH):
                nc.scalar.copy(v_t[tb][:, ec * QCH:(ec + 1) * QCH], psA[ec][:])

        p1.close()

        p2 = ExitStack()
        wT_pool = p2.enter_context(tc.tile_pool(name=f"pwT{sfx}", bufs=1, side="right"))
        wT_t = [wT_pool.tile([P, T], BF16, tag=f"wT{i}", name=f"wT{i}{sfx}") for i in range(NT)]

        # scores: weight kT[e][:,kt] loaded once per (kt,e), reused for the
        # computed q-chunks
        for kt in range(NT):
            qcd = kt // 4
            psA = {qc: psum.tile([P, QCH], F32, tag="mm", name=f"ps_s{kt}_{qc}{sfx}")
                   for qc in range(qcd, NQC)}
            for e in range(NE):
                for qc in range(qcd, NQC):
                    nc.tensor.matmul(
                        psA[qc][:],
                        kT_t[e][:, kt * P:(kt + 1) * P],
                        qT_t[e][:, qc * QCH:(qc + 1) * QCH],
                        start=(e == 0), stop=(e == NE - 1),
                    )
            for qc in range(qcd, NQC):
                wslice = wT_t[kt][:, qc * QCH:(qc + 1) * QCH]
                acc = parts_t[:, kt, qc:qc + 1]
                if qc == qcd:
                    dv = kt % 4
                    stg = stage.tile([P, QCH], F32, tag="stg", name=f"stg{kt}{sfx}")
                    nc.vector.tensor_add(stg[:], psA[qc][:], mask_t[:, dv, :])
                    nc.scalar.activation(wslice, stg[:], Exp, bias=0.0,
                                         scale=SCALE, accum_out=acc)
                else:
                    nc.scalar.activation(wslice, psA[qc][:], Exp, bias=0.0,
                                         scale=SCALE, accum_out=acc)
            nc.vector.reduce_sum(denom_t[:, kt:kt + 1], parts_t[:, kt, qcd:NQC],
                                 axis=mybir.AxisListType.X)
            nc.vector.reciprocal(recip_t[:, kt:kt + 1], denom_t[:, kt:kt + 1])
            nc.vector.tensor_scalar_mul(v_t[kt][:], v_t[kt][:], recip_t[:, kt:kt + 1])

        pkq.close()

        p3 = ExitStack()
        outT_pool = p3.enter_context(tc.tile_pool(name=f"poutT{sfx}", bufs=1))
        outT_t = [outT_pool.tile([P, T], BF16, tag=f"oT{i}", name=f"oT{i}{sfx}") for i in range(NE)]

        # AV: weight v_t[kt][:,eb] loaded once per (eb,kt), reused for the
        # q-chunks that include kt
        for eb in range(NE):
            psA = [psum.tile([P, QCH], F32, tag="mm", name=f"ps_o{eb}_{qc}{sfx}")
                   for qc in range(NQC)]
            for kt in range(NT):
                for qc in range(NQC):
                    if kt >= (qc + 1) * 4:
                        continue
                    nc.tensor.matmul(
                        psA[qc][:],
                        v_t[kt][:, eb * P:(eb + 1) * P],
                        wT_t[kt][:, qc * QCH:(qc + 1) * QCH],
                        start=(kt == 0), stop=(kt == (qc + 1) * 4 - 1),
                    )
            for qc in range(NQC):
                nc.scalar.copy(outT_t[eb][:, qc * QCH:(qc + 1) * QCH], psA[qc][:])

        p2.close()

        p4 = ExitStack()
        ro_pool = p4.enter_context(tc.tile_pool(name=f"pro{sfx}", bufs=2))
        ostg_pool = p4.enter_context(tc.tile_pool(name=f"postg{sfx}", bufs=4))

        # readout: weight outT[e][:,tb] loaded once per (tb,e), reused for a
        # group of 4 vocab chunks
        VGRP = 4
        for vg in range(NVC // VGRP):
            wro_t = ro_pool.tile([P, NE, VGRP, VCH], BF16, tag="wro", name=f"wro{vg}{sfx}")
            for e in range(NE):
                for vq in range(VGRP):
                    vc = vg * VGRP + vq
                    nc.sync.dma_start(wro_t[:, e, vq, :],
                                      wro_d[e, :, vc * VCH:(vc + 1) * VCH])
            for tb in range(NT):
                psA = [psum.tile([P, VCH], F32, tag="mm", name=f"ps_r{vg}_{tb}_{vq}{sfx}")
                       for vq in range(VGRP)]
                for e in range(NE):
                    for vq in range(VGRP):
                        nc.tensor.matmul(
                            psA[vq][:],
                            outT_t[e][:, tb * P:(tb + 1) * P],
                            wro_t[:, e, vq, :],
                            start=(e == 0), stop=(e == NE - 1),
                        )
                for vq in range(VGRP):
                    vc = vg * VGRP + vq
                    stg = ostg_pool.tile([P, VCH], F32, tag="ostg",
                                         name=f"ostg{vc}_{tb}{sfx}")
                    if vq % 2 == 0:
                        nc.vector.tensor_copy(stg[:], psA[vq][:])
                    else:
                        nc.scalar.copy(stg[:], psA[vq][:])
                    nc.sync.dma_start(out_d[tb, :, vc * VCH:(vc + 1) * VCH], stg[:])

        p4.close()
        p3.close()


def _emit_body_rep3(tc, nc, aps, sfx):
    """rep2 + fp8 DoubleRow for k/q projections and score matmuls (PE halved
    on those phases) + bf16 logits staging (halves output DMA).  v / AV /
    readout stay bf16 — fp8 there would push rel_err past the 2e-2 gate."""
    xT_d, xT8_d, wk8_d, wq8_d, wvp_d, wro_d, mask4_d, out_d = aps
    Exp = mybir.ActivationFunctionType.Exp
    Copy = mybir.ActivationFunctionType.Copy
    NEP = NE // 2  # fp8 DoubleRow pairs along the contraction dim

    with ExitStack() as root:
        misc = root.enter_context(tc.tile_pool(name=f"misc{sfx}", bufs=1))
        psum = root.enter_context(tc.tile_pool(name=f"psum{sfx}", bufs=8, space="PSUM"))
        stage = root.enter_context(tc.tile_pool(name=f"stage{sfx}", bufs=2))

        mask_t = misc.tile([P, NQC, QCH], F32, tag="mask", name=f"mask_t{sfx}")
        nc.sync.dma_start(mask_t[:], mask4_d[:])
        parts_t = misc.tile([P, NT, NQC], F32, tag="parts", name=f"parts_t{sfx}")
        denom_t = misc.tile([P, NT], F32, tag="denom", name=f"denom_t{sfx}")
        recip_t = misc.tile([P, NT], F32, tag="recip", name=f"recip_t{sfx}")

        pv = root.enter_context(tc.tile_pool(name=f"pv{sfx}", bufs=1))
        v_t = [pv.tile([P, E], BF16, tag=f"v{i}", name=f"v{i}{sfx}") for i in range(NT)]

        pkq = ExitStack()
        kq_pool = pkq.enter_context(tc.tile_pool(name=f"pkq{sfx}", bufs=1))
        kT8_t = kq_pool.tile([P, NE, T], F8, tag="kT8", name=f"kT8{sfx}")
        qT8_t = kq_pool.tile([P, NE, T], F8, tag="qT8", name=f"qT8{sfx}")

        p1 = ExitStack()
        x_pool = p1.enter_context(tc.tile_pool(name=f"px{sfx}", bufs=1))
        w_pool = p1.enter_context(tc.tile_pool(name=f"pw{sfx}", bufs=1))

        xT_t = [x_pool.tile([P, T], BF16, tag=f"xT{i}", name=f"xT{i}{sfx}") for i in range(NE)]
        for e in range(NE):
            nc.sync.dma_start(xT_t[e][:], xT_d[e])
        xT8_t = x_pool.tile([P, NE, T], F8, tag="xT8", name=f"xT8{sfx}")
        nc.sync.dma_start(xT8_t[:], xT8_d[:])

        wk8_t = w_pool.tile([P, NE, E], F8, tag="wk8", name=f"wk8_t{sfx}")
        nc.sync.dma_start(wk8_t[:], wk8_d[:])
        wq8_t = w_pool.tile([P, NE, E], F8, tag="wq8", name=f"wq8_t{sfx}")
        nc.sync.dma_start(wq8_t[:], wq8_d[:])
        wv_t = w_pool.tile([P, NE, E], BF16, tag="wv", name=f"wv_t{sfx}")
        nc.sync.dma_start(wv_t[:], wvp_d[:])

        # k/q projections, fp8 DoubleRow: stationary w-pair reused over 4
        # t-chunks; psum = x@W * (XS*WS), stored to fp8 at QS.
        for (w8_t, dst) in ((wk8_t, kT8_t), (wq8_t, qT8_t)):
            for eo in range(NE):
                psA = [psum.tile([P, QCH], F32, tag="mm", name=f"ps_{eo}_{t_}{sfx}")
                       for t_ in range(NQC)]
                for ep in range(NEP):
                    for tch in range(NQC):
                        nc.tensor.matmul(
                            psA[tch][:],
                            w8_t[:, 2 * ep:2 * ep + 2, eo * P:(eo + 1) * P],
                            xT8_t[:, 2 * ep:2 * ep + 2, tch * QCH:(tch + 1) * QCH],
                            start=(ep == 0), stop=(ep == NEP - 1),
                            perf_mode=DR,
                        )
                for tch in range(NQC):
                    nc.scalar.activation(
                        dst[:, eo, tch * QCH:(tch + 1) * QCH], psA[tch][:],
                        Copy, bias=0.0, scale=S_PROJ)

        # V projection (bf16): stationary xT[:, tb] reused over 2 e-chunks
        for tb in range(NT):
            psA = [psum.tile([P, QCH], F32, tag="mm", name=f"ps_v{tb}_{ec}{sfx}")
                   for ec in range(E // QCH)]
            for e in range(NE):
                for ec in range(E // QCH):
                    nc.tensor.matmul(
                        psA[ec][:],
                        xT_t[e][:, tb * P:(tb + 1) * P],
                        wv_t[:, e, ec * QCH:(ec + 1) * QCH],
                        start=(e == 0), stop=(e == NE - 1),
                    )
            for ec in range(E // QCH):
                nc.scalar.copy(v_t[tb][:, ec * QCH:(ec + 1) * QCH], psA[ec][:])

        p1.close()

        p2 = ExitStack()
        wT_pool = p2.enter_context(tc.tile_pool(name=f"pwT{sfx}", bufs=1, side="right"))
        wT_t = [wT_pool.tile([P, T], BF16, tag=f"wT{i}", name=f"wT{i}{sfx}") for i in range(NT)]

        # scores, fp8 DoubleRow: psum = q.k * QS^2; exp scale folds it back
        for kt in range(NT):
            qcd = kt // 4
            psA = {qc: psum.tile([P, QCH], F32, tag="mm", name=f"ps_s{kt}_{qc}{sfx}")
                   for qc in range(qcd, NQC)}
            for ep in range(NEP):
                for qc in range(qcd, NQC):
                    nc.tensor.matmul(
                        psA[qc][:],
                        kT8_t[:, 2 * ep:2 * ep + 2, kt * P:(kt + 1) * P],
                        qT8_t[:, 2 * ep:2 * ep + 2, qc * QCH:(qc + 1) * QCH],
                        start=(ep == 0), stop=(ep == NEP - 1),
                        perf_mode=DR,
                    )
            for qc in range(qcd, NQC):
                wslice = wT_t[kt][:, qc * QCH:(qc + 1) * QCH]
                acc = parts_t[:, kt, qc:qc + 1]
                if qc == qcd:
                    dv = kt % 4
                    stg = stage.tile([P, QCH], F32, tag="stg", name=f"stg{kt}{sfx}")
                    nc.vector.tensor_add(stg[:], psA[qc][:], mask_t[:, dv, :])
                    nc.scalar.activation(wslice, stg[:], Exp, bias=0.0,
                                         scale=S_EXP, accum_out=acc)
                else:
                    nc.scalar.activation(wslice, psA[qc][:], Exp, bias=0.0,
                                         scale=S_EXP, accum_out=acc)
            nc.vector.reduce_sum(denom_t[:, kt:kt + 1], parts_t[:, kt, qcd:NQC],
                                 axis=mybir.AxisListType.X)
            nc.vector.reciprocal(recip_t[:, kt:kt + 1], denom_t[:, kt:kt + 1])
            nc.vector.tensor_scalar_mul(v_t[kt][:], v_t[kt][:], recip_t[:, kt:kt + 1])

        pkq.close()

        p3 = ExitStack()
        outT_pool = p3.enter_context(tc.tile_pool(name=f"poutT{sfx}", bufs=1))
        outT_t = [outT_pool.tile([P, T], BF16, tag=f"oT{i}", name=f"oT{i}{sfx}") for i in range(NE)]

        # AV (bf16): stationary v_t[kt][:, eb] reused over valid q-chunks
        for eb in range(NE):
            psA = [psum.tile([P, QCH], F32, tag="mm", name=f"ps_o{eb}_{qc}{sfx}")
                   for qc in range(NQC)]
            for kt in range(NT):
                for qc in range(NQC):
                    if kt >= (qc + 1) * 4:
                        continue
                    nc.tensor.matmul(
                        psA[qc][:],
                        v_t[kt][:, eb * P:(eb + 1) * P],
                        wT_t[kt][:, qc * QCH:(qc + 1) * QCH],
                        start=(kt == 0), stop=(kt == (qc + 1) * 4 - 1),
                    )
            for qc in range(NQC):
                nc.scalar.copy(outT_t[eb][:, qc * QCH:(qc + 1) * QCH], psA[qc][:])

        p2.close()

        p4 = ExitStack()
        ro_pool = p4.enter_context(tc.tile_pool(name=f"pro{sfx}", bufs=2))
        ostg_pool = p4.enter_context(tc.tile_pool(name=f"postg{sfx}", bufs=4))

        # readout (bf16): stationary outT[e][:, tb] reused over 4 vocab chunks
        VGRP = 4
        for vg in range(NVC // VGRP):
            wro_t = ro_pool.tile([P, NE, VGRP, VCH], BF16, tag="wro", name=f"wro{vg}{sfx}")
            for e in range(NE):
                for vq in range(VGRP):
                    vc = vg * VGRP + vq
                    nc.sync.dma_start(wro_t[:, e, vq, :],
                                      wro_d[e, :, vc * VCH:(vc + 1) * VCH])
            for tb in range(NT):
                psA = [psum.tile([P, VCH], F32, tag="mm", name=f"ps_r{vg}_{tb}_{vq}{sfx}")
                       for vq in range(VGRP)]
                for e in range(NE):
                    for vq in range(VGRP):
                        nc.tensor.matmul(
                            psA[vq][:],
                            outT_t[e][:, tb * P:(tb + 1) * P],
                            wro_t[:, e, vq, :],
                            start=(e == 0), stop=(e == NE - 1),
                        )
                for vq in range(VGRP):
                    vc = vg * VGRP + vq
                    stg = ostg_pool.tile([P, VCH], BF16, tag="ostg",
                                         name=f"ostg{vc}_{tb}{sfx}")
                    if vq % 2 == 0:
                        nc.vector.tensor_copy(stg[:], psA[vq][:])
                    else:
                        nc.scalar.copy(stg[:], psA[vq][:])
                    nc.sync.dma_start(out_d[tb, :, vc * VCH:(vc + 1) * VCH], stg[:])

        p4.close()
        p3.close()


def _emit_body_rep4(tc, nc, aps, sfx):
    """rep3 + error-compensated fp8 readout.

    The attention output o and Wro are each split into fp8 (high, low)
    parts at a shared scale: a ~= a_h + a_l with a_l the rounding residual
    (stored directly in fp8 — residuals land in the normal/subnormal range
    where their own rounding error is ~0.4% of a, i.e. negligible).  Then

        logits ~= o_h@W_h + o_h@W_l + o_l@W_h      (o_l@W_l dropped)

    and all three terms share one PSUM accumulation at the same scale, so
    the epilogue is the same single scaled copy as bf16.  12 DoubleRow
    matmuls replace 8 bf16 matmuls per output tile: 25% fewer PE cycles
    on the dominant phase at full bf16-level accuracy."""
    xT_d, xT8_d, wk8_d, wq8_d, wvp_d, wro8h_d, wro8l_d, mask4_d, out_d = aps
    Exp = mybir.ActivationFunctionType.Exp
    Copy = mybir.ActivationFunctionType.Copy
    NEP = NE // 2

    with ExitStack() as root:
        misc = root.enter_context(tc.tile_pool(name=f"misc{sfx}", bufs=1))
        psum = root.enter_context(tc.tile_pool(name=f"psum{sfx}", bufs=8, space="PSUM"))
        stage = root.enter_context(tc.tile_pool(name=f"stage{sfx}", bufs=2))

        mask_t = misc.tile([P, NQC, QCH], F32, tag="mask", name=f"mask_t{sfx}")
        nc.sync.dma_start(mask_t[:], mask4_d[:])
        parts_t = misc.tile([P, NT, NQC], F32, tag="parts", name=f"parts_t{sfx}")
        denom_t = misc.tile([P, NT], F32, tag="denom", name=f"denom_t{sfx}")
        recip_t = misc.tile([P, NT], F32, tag="recip", name=f"recip_t{sfx}")
        recs_t = misc.tile([P, NT], F32, tag="recs", name=f"recs_t{sfx}")

        pv = root.enter_context(tc.tile_pool(name=f"pv{sfx}", bufs=1))
        v_t = [pv.tile([P, E], BF16, tag=f"v{i}", name=f"v{i}{sfx}") for i in range(NT)]

        pkq = ExitStack()
        kq_pool = pkq.enter_context(tc.tile_pool(name=f"pkq{sfx}", bufs=1))
        kT8_t = kq_pool.tile([P, NE, T], F8, tag="kT8", name=f"kT8{sfx}")
        qT8_t = kq_pool.tile([P, NE, T], F8, tag="qT8", name=f"qT8{sfx}")

        p1 = ExitStack()
        x_pool = p1.enter_context(tc.tile_pool(name=f"px{sfx}", bufs=1))
        w_pool = p1.enter_context(tc.tile_pool(name=f"pw{sfx}", bufs=1))

        xT_t = [x_pool.tile([P, T], BF16, tag=f"xT{i}", name=f"xT{i}{sfx}") for i in range(NE)]
        for e in range(NE):
            nc.sync.dma_start(xT_t[e][:], xT_d[e])
        xT8_t = x_pool.tile([P, NE, T], F8, tag="xT8", name=f"xT8{sfx}")
        nc.sync.dma_start(xT8_t[:], xT8_d[:])

        wk8_t = w_pool.tile([P, NE, E], F8, tag="wk8", name=f"wk8_t{sfx}")
        nc.sync.dma_start(wk8_t[:], wk8_d[:])
        wq8_t = w_pool.tile([P, NE, E], F8, tag="wq8", name=f"wq8_t{sfx}")
        nc.sync.dma_start(wq8_t[:], wq8_d[:])
        wv_t = w_pool.tile([P, NE, E], BF16, tag="wv", name=f"wv_t{sfx}")
        nc.sync.dma_start(wv_t[:], wvp_d[:])

        for (w8_t, dst) in ((wk8_t, kT8_t), (wq8_t, qT8_t)):
            for eo in range(NE):
                psA = [psum.tile([P, QCH], F32, tag="mm", name=f"ps_{eo}_{t_}{sfx}")
                       for t_ in range(NQC)]
                for ep in range(NEP):
                    for tch in range(NQC):
                        nc.tensor.matmul(
                            psA[tch][:],
                            w8_t[:, 2 * ep:2 * ep + 2, eo * P:(eo + 1) * P],
                            xT8_t[:, 2 * ep:2 * ep + 2, tch * QCH:(tch + 1) * QCH],
                            start=(ep == 0), stop=(ep == NEP - 1),
                            perf_mode=DR,
                        )
                for tch in range(NQC):
                    nc.scalar.activation(
                        dst[:, eo, tch * QCH:(tch + 1) * QCH], psA[tch][:],
                        Copy, bias=0.0, scale=S_PROJ)

        for tb in range(NT):
            psA = [psum.tile([P, QCH], F32, tag="mm", name=f"ps_v{tb}_{ec}{sfx}")
                   for ec in range(E // QCH)]
            for e in range(NE):
                for ec in range(E // QCH):
                    nc.tensor.matmul(
                        psA[ec][:],
                        xT_t[e][:, tb * P:(tb + 1) * P],
                        wv_t[:, e, ec * QCH:(ec + 1) * QCH],
                        start=(e == 0), stop=(e == NE - 1),
                    )
            for ec in range(E // QCH):
                nc.scalar.copy(v_t[tb][:, ec * QCH:(ec + 1) * QCH], psA[ec][:])

        p1.close()

        p2 = ExitStack()
        wT_pool = p2.enter_context(tc.tile_pool(name=f"pwT{sfx}", bufs=1, side="right"))
        wT_t = [wT_pool.tile([P, T], BF16, tag=f"wT{i}", name=f"wT{i}{sfx}") for i in range(NT)]

        for kt in range(NT):
            qcd = kt // 4
            psA = {qc: psum.tile([P, QCH], F32, tag="mm", name=f"ps_s{kt}_{qc}{sfx}")
                   for qc in range(qcd, NQC)}
            for ep in range(NEP):
                for qc in range(qcd, NQC):
                    nc.tensor.matmul(
                        psA[qc][:],
                        kT8_t[:, 2 * ep:2 * ep + 2, kt * P:(kt + 1) * P],
                        qT8_t[:, 2 * ep:2 * ep + 2, qc * QCH:(qc + 1) * QCH],
                        start=(ep == 0), stop=(ep == NEP - 1),
                        perf_mode=DR,
                    )
            for qc in range(qcd, NQC):
                wslice = wT_t[kt][:, qc * QCH:(qc + 1) * QCH]
                acc = parts_t[:, kt, qc:qc + 1]
                if qc == qcd:
                    dv = kt % 4
                    stg = stage.tile([P, QCH], F32, tag="stg", name=f"stg{kt}{sfx}")
                    nc.vector.tensor_add(stg[:], psA[qc][:], mask_t[:, dv, :])
                    nc.scalar.activation(wslice, stg[:], Exp, bias=0.0,
                                         scale=S_EXP, accum_out=acc)
                else:
                    nc.scalar.activation(wslice, psA[qc][:], Exp, bias=0.0,
                                         scale=S_EXP, accum_out=acc)
            nc.vector.reduce_sum(denom_t[:, kt:kt + 1], parts_t[:, kt, qcd:NQC],
                                 axis=mybir.AxisListType.X)
            nc.vector.reciprocal(recip_t[:, kt:kt + 1], denom_t[:, kt:kt + 1])
            # fold the fp8 storage scale OS for o into the softmax denominator
            nc.scalar.activation(recs_t[:, kt:kt + 1], recip_t[:, kt:kt + 1],
                                 Copy, bias=0.0, scale=OS)
            nc.vector.tensor_scalar_mul(v_t[kt][:], v_t[kt][:], recs_t[:, kt:kt + 1])

        pkq.close()

        p3 = ExitStack()
        outT_pool = p3.enter_context(tc.tile_pool(name=f"poutT{sfx}", bufs=1))
        oh_t = outT_pool.tile([P, NE, T], F8, tag="oh", name=f"oh{sfx}")
        ol_t = outT_pool.tile([P, NE, T], F8, tag="ol", name=f"ol{sfx}")

        # AV (bf16, pre-scaled by OS): split PSUM into fp8 high + residual
        for eb in range(NE):
            psA = [psum.tile([P, QCH], F32, tag="mm", name=f"ps_o{eb}_{qc}{sfx}")
                   for qc in range(NQC)]
            for kt in range(NT):
                for qc in range(NQC):
                    if kt >= (qc + 1) * 4:
                        continue
                    nc.tensor.matmul(
                        psA[qc][:],
                        v_t[kt][:, eb * P:(eb + 1) * P],
                        wT_t[kt][:, qc * QCH:(qc + 1) * QCH],
                        start=(kt == 0), stop=(kt == (qc + 1) * 4 - 1),
                    )
            for qc in range(NQC):
                ohs = oh_t[:, eb, qc * QCH:(qc + 1) * QCH]
                ols = ol_t[:, eb, qc * QCH:(qc + 1) * QCH]
                nc.scalar.copy(ohs, psA[qc][:])
                nc.vector.tensor_sub(ols, psA[qc][:], ohs)

        p2.close()

        p4 = ExitStack()
        ro_pool = p4.enter_context(tc.tile_pool(name=f"pro{sfx}", bufs=2))
        ostg_pool = p4.enter_context(tc.tile_pool(name=f"postg{sfx}", bufs=4))

        VGRP = 4
        for vg in range(NVC // VGRP):
            wh_t = ro_pool.tile([P, NE, VGRP, VCH], F8, tag="wh", name=f"wh{vg}{sfx}")
            wl_t = ro_pool.tile([P, NE, VGRP, VCH], F8, tag="wl", name=f"wl{vg}{sfx}")
            for e in range(NE):
                nc.sync.dma_start(wh_t[:, e, :, :],
                                  wro8h_d[:, e, vg * VGRP:(vg + 1) * VGRP, :])
                nc.sync.dma_start(wl_t[:, e, :, :],
                                  wro8l_d[:, e, vg * VGRP:(vg + 1) * VGRP, :])
            for tb in range(NT):
                psA = [psum.tile([P, VCH], F32, tag="mm", name=f"ps_r{vg}_{tb}_{vq}{sfx}")
                       for vq in range(VGRP)]
                tbs = slice(tb * P, (tb + 1) * P)
                # o_h stationary: main + W-residual terms share the weights
                for ep in range(NEP):
                    ohp = oh_t[:, 2 * ep:2 * ep + 2, tbs]
                    for vq in range(VGRP):
                        nc.tensor.matmul(
                            psA[vq][:], ohp, wh_t[:, 2 * ep:2 * ep + 2, vq, :],
                            start=(ep == 0), stop=False, perf_mode=DR)
                    for vq in range(VGRP):
                        nc.tensor.matmul(
                            psA[vq][:], ohp, wl_t[:, 2 * ep:2 * ep + 2, vq, :],
                            start=False, stop=False, perf_mode=DR)
                # o_l stationary: o-residual term
                for ep in range(NEP):
                    olp = ol_t[:, 2 * ep:2 * ep + 2, tbs]
                    for vq in range(VGRP):
                        nc.tensor.matmul(
                            psA[vq][:], olp, wh_t[:, 2 * ep:2 * ep + 2, vq, :],
                            start=False, stop=(ep == NEP - 1), perf_mode=DR)
                for vq in range(VGRP):
                    vc = vg * VGRP + vq
                    stg = ostg_pool.tile([P, VCH], BF16, tag="ostg",
                                         name=f"ostg{vc}_{tb}{sfx}")
                    if vq % 2 == 0:
                        nc.vector.tensor_scalar_mul(stg[:], psA[vq][:], S_RO)
                    else:
                        nc.scalar.activation(stg[:], psA[vq][:], Copy,
                                             bias=0.0, scale=S_RO)
                    nc.sync.dma_start(out_d[tb, :, vc * VCH:(vc + 1) * VCH], stg[:])

        p4.close()
        p3.close()


_EMITTERS = {"cc": _emit_body_cc, "rep": _emit_body_rep, "rep2": _emit_body_rep2,
             "rep3": _emit_body_rep3, "rep4": _emit_body_rep4}


def _build_program(mode=MODE, reps=1):
    nc = bacc.Bacc("TRN2", target_bir_lowering=False, debug=False, num_devices=8)

    if mode in ("rep3", "rep4"):
        xT_d = nc.dram_tensor("xT", [NE, P, T], BF16, kind="ExternalInput").ap()
        xT8_d = nc.dram_tensor("xT8", [P, NE, T], F8, kind="ExternalInput").ap()
        wk8_d = nc.dram_tensor("wk8", [P, NE, E], F8, kind="ExternalInput").ap()
        wq8_d = nc.dram_tensor("wq8", [P, NE, E], F8, kind="ExternalInput").ap()
        wvp_d = nc.dram_tensor("wvp", [P, NE, E], BF16, kind="ExternalInput").ap()
        mask_d = nc.dram_tensor("mask4", [P, NQC, QCH], F32, kind="ExternalInput").ap()
        out_d = nc.dram_tensor("logits", [NT, P, VS], BF16, kind="ExternalOutput").ap()
        if mode == "rep3":
            wro_d = nc.dram_tensor("wro", [NE, P, VS], BF16, kind="ExternalInput").ap()
            aps = (xT_d, xT8_d, wk8_d, wq8_d, wvp_d, wro_d, mask_d, out_d)
        else:
            wro8h_d = nc.dram_tensor("wro8h", [P, NE, NVC, VCH], F8,
                                     kind="ExternalInput").ap()
            wro8l_d = nc.dram_tensor("wro8l", [P, NE, NVC, VCH], F8,
                                     kind="ExternalInput").ap()
            aps = (xT_d, xT8_d, wk8_d, wq8_d, wvp_d, wro8h_d, wro8l_d,
                   mask_d, out_d)
        emit = _EMITTERS[mode]
        with tile.TileContext(nc) as tc:
            for r in range(reps):
                emit(tc, nc, aps, f"_r{r}" if reps > 1 else "")
        nc.compile()
        return nc

    xT_d = nc.dram_tensor("xT", [NE, P, T], BF16, kind="ExternalInput").ap()
    xTk_d = (nc.dram_tensor("xTk", [NE, P, TK], BF16, kind="ExternalInput").ap()
             if mode == "cc" else None)
    wk_d = nc.dram_tensor("wk", [NE, P, E], BF16, kind="ExternalInput").ap()
    wq_d = nc.dram_tensor("wq", [NE, P, E], BF16, kind="ExternalInput").ap()
    wv_d = nc.dram_tensor("wv", [NE, P, E], BF16, kind="ExternalInput").ap()
    wro_d = nc.dram_tensor("wro", [NE, P, VS], BF16, kind="ExternalInput").ap()
    if mode == "cc":
        mask_d = nc.dram_tensor("mask", [P, QCH], F32, kind="ExternalInput").ap()
    else:
        mask_d = nc.dram_tensor("mask4", [P, NQC, QCH], F32, kind="ExternalInput").ap()
    out_d = nc.dram_tensor("logits", [NT, P, VS], F32, kind="ExternalOutput").ap()

    aps = (xT_d, xTk_d, wk_d, wq_d, wv_d, wro_d, mask_d, out_d)
    emit = _EMITTERS[mode]

    with tile.TileContext(nc) as tc:
        for r in range(reps):
            emit(tc, nc, aps, f"_r{r}" if reps > 1 else "")

    nc.compile()
    return nc


def _get_nc():
    if "nc" not in _CACHE:
        _CACHE["nc"] = _build_program()
    return _CACHE["nc"]


def _make_in_maps_rep3(X, emb_table, pos_table, Wk, Wq, Wv, Wro):
    bf = ml_dtypes.bfloat16
    f8 = ml_dtypes.float8_e4m3
    X = np.asarray(X)
    emb_table = np.asarray(emb_table, np.float32)
    pos_table = np.asarray(pos_table, np.float32)

    x = emb_table[X] + pos_table[None, :, :]            # [B, T, E] f32

    wk8 = np.ascontiguousarray(
        np.asarray(Wk, np.float32).reshape(NE, P, E).transpose(1, 0, 2) * WS
    ).astype(f8)
    wq8 = np.ascontiguousarray(
        np.asarray(Wq, np.float32).reshape(NE, P, E).transpose(1, 0, 2) * WS
    ).astype(f8)
    wvp = np.ascontiguousarray(
        np.asarray(Wv, np.float32).reshape(NE, P, E).transpose(1, 0, 2)
    ).astype(bf)

    Wro = np.asarray(Wro, np.float32)
    wro_s, wro_h, wro_l = [], [], []
    for s in range(VSPLIT):
        sl = Wro[:, s * VS:(s + 1) * VS].reshape(NE, P, VS)
        if MODE == "rep4":
            scaled = np.ascontiguousarray(
                sl.transpose(1, 0, 2) * ROW_S)              # [P, NE, VS]
            h = scaled.astype(f8)
            l = (scaled - h.astype(np.float32)).astype(f8)
            wro_h.append(np.ascontiguousarray(h.reshape(P, NE, NVC, VCH)))
            wro_l.append(np.ascontiguousarray(l.reshape(P, NE, NVC, VCH)))
        else:
            wro_s.append(np.ascontiguousarray(sl).astype(bf))

    xT_b, xT8_b = [], []
    for b in range(B):
        xt = np.ascontiguousarray(x[b].T)                       # [E, T] f32
        xT_b.append(xt.reshape(NE, P, T).astype(bf))
        xT8_b.append(np.ascontiguousarray(
            xt.reshape(NE, P, T).transpose(1, 0, 2) * XS).astype(f8))

    p_idx = np.arange(P)[:, None]
    c_idx = np.arange(QCH)[None, :]
    masks = [
        np.where(c_idx < dv * P + p_idx, MASK_R3, 0.0).astype(np.float32)
        for dv in range(VSPLIT)
    ]
    mask4 = np.stack(masks, axis=1)                             # [P, NQC, QCH]

    in_maps = []
    for c in range(8):
        b, dv = divmod(c, VSPLIT)
        m = {
            "xT": xT_b[b],
            "xT8": xT8_b[b],
            "wk8": wk8, "wq8": wq8, "wvp": wvp,
            "mask4": mask4,
        }
        if MODE == "rep4":
            m["wro8h"] = wro_h[dv]
            m["wro8l"] = wro_l[dv]
        else:
            m["wro"] = wro_s[dv]
        in_maps.append(m)
    return in_maps


def _make_in_maps(X, emb_table, pos_table, Wk, Wq, Wv, Wro):
    if MODE in ("rep3", "rep4"):
        return _make_in_maps_rep3(X, emb_table, pos_table, Wk, Wq, Wv, Wro)
    bf = ml_dtypes.bfloat16
    X = np.asarray(X)
    emb_table = np.asarray(emb_table, np.float32)
    pos_table = np.asarray(pos_table, np.float32)

    # host-side embedding gather + positional add (0.03% of model FLOPs)
    x = emb_table[X] + pos_table[None, :, :]            # [B, T, E] f32

    wk = np.ascontiguousarray(np.asarray(Wk, np.float32).reshape(NE, P, E)).astype(bf)
    wq = np.ascontiguousarray(np.asarray(Wq, np.float32).reshape(NE, P, E)).astype(bf)
    wv = np.ascontiguousarray(np.asarray(Wv, np.float32).reshape(NE, P, E)).astype(bf)

    Wro = np.asarray(Wro, np.float32)
    wro_s = []
    for s in range(VSPLIT):
        sl = Wro[:, s * VS:(s + 1) * VS].reshape(NE, P, VS)
        wro_s.append(np.ascontiguousarray(sl).astype(bf))

    xT_b, xTk_b = [], []
    for b in range(B):
        xt = np.ascontiguousarray(x[b].T)                       # [E, T] f32
        xT_b.append(xt.reshape(NE, P, T).astype(bf))
        per_dv = []
        for dv in range(VSPLIT):
            cols = np.concatenate(
                [xt[:, (dv + 4 * j) * P:(dv + 4 * j + 1) * P] for j in range(KL)],
                axis=1,
            )                                                   # [E, TK]
            per_dv.append(np.ascontiguousarray(cols).reshape(NE, P, TK).astype(bf))
        xTk_b.append(per_dv)

    # staircase masks: masked iff col < dv*128 + p (diag chunk of k-tile dv+4j)
    p_idx = np.arange(P)[:, None]
    c_idx = np.arange(QCH)[None, :]
    masks = [
        np.where(c_idx < dv * P + p_idx, MASK_VAL, 0.0).astype(np.float32)
        for dv in range(VSPLIT)
    ]
    mask4 = np.stack(masks, axis=1)                             # [P, NQC, QCH]

    in_maps = []
    for c in range(8):
        b, dv = divmod(c, VSPLIT)
        in_maps.append({
            "xT": xT_b[b],
            "xTk": xTk_b[b][dv],
            "wk": wk, "wq": wq, "wv": wv,
            "wro": wro_s[dv],
            "mask": masks[dv],
            "mask4": mask4,
        })
    return in_maps


def run_on_device(in_maps, trace=False, **kw):
    nc = _get_nc()
    return run_bass_kernel_spmd(nc, in_maps, core_ids=list(range(8)), trace=trace, **kw)


def _unshard(results):
    logits = np.empty((B, T, VOC), np.float32)
    for c in range(8):
        b, s = divmod(c, VSPLIT)
        logits[b, :, s * VS:(s + 1) * VS] = (
            results[c]["logits"].reshape(T, VS).astype(np.float32))
    return logits


def kernel(X, emb_table, pos_table, Wk, Wq, Wv, Wro, bro):
    in_maps = _make_in_maps(X, emb_table, pos_table, Wk, Wq, Wv, Wro)
    _CACHE["in_maps"] = in_maps

    res = run_on_device(in_maps, trace=False)
    _CACHE["last_results"] = res

    logits = _unshard(res.results)

    bro = np.asarray(bro, np.float32)
    if np.any(bro):
        logits += bro
    return logits



# revision 13
# speedup vs baseline: 1.7657x; 1.7657x over previous
"""Trainium2 Bass kernel for nn_BigramLanguageModel (dense transformer block).

Reference computation (B=2, T=2048, E=1024, V=32000):
    x      = emb_table[X] + pos_table                       # [B,T,E]
    k,q,v  = x@Wk, x@Wq, x@Wv                               # [B,T,E]
    w      = (q @ k^T) / sqrt(E), causal mask (tril)        # [B,T,T]
    w      = softmax(w, axis=1)          # QUIRK: over the *query* axis
    out    = w @ v                                          # [B,T,E]
    logits = out @ Wro + bro                                # [B,T,V]

Sharding: 8 cores = 2 (batch) x 4 (vocab slices of 8000 for the readout
matmul, which dominates FLOPs).  Each core computes the full attention
for its batch (replicated within the 4-core group — measured cheaper
than any collective-based split at this size).

Device-side layout: scores are computed transposed, wT[k,q], so the
softmax-over-q (the model's faithful quirk) runs along the free axis.
The softmax denominator depends only on k, so it is folded into V
(V' = V/denom[k]) and the attention output is produced directly in
outT[e,q] layout — exactly the lhsT layout the readout matmul wants.
Causal masking uses the block structure: chunks with q_end <= k0 are
never computed nor read; only the single diagonal 512-chunk per k-tile
needs an additive staircase mask.

Precision/speed split (mode "rep3", measured fastest):
  - k/q projections and score matmuls run in fp8e4m3 with DoubleRow
    perf mode (one instruction contracts 2 K-blocks -> half the PE
    instructions; score noise is crushed by the softmax).
  - v projection, AV, and the readout stay bf16: fp8 there leaks ~2-4%
    relative error straight into the logits (measured 4e-2, over the
    2e-2 gate).
  - logits are written bf16 (halves the dominant output-write DMA) and
    upcast to f32 on the host.
Measured on HW (slope of NEFF-internal repeats): rep2 all-bf16/f32-out
~833 us/rep; rep3 ~690 us/rep; an error-compensated full-fp8 readout
("rep4", 12 DoubleRow insts vs 8 bf16 per tile) measured ~908 us/rep —
a DoubleRow instruction costs the same time as a bf16 one on TRN2, so
12 > 8 loses despite the fp8 rate.
"""

import sys

if "/opt/trn_rl_repo" not in sys.path:
    sys.path.insert(0, "/opt/trn_rl_repo")

from contextlib import ExitStack

import numpy as np
import ml_dtypes

import concourse.bass as bass
import concourse.tile as tile
from concourse import bacc, mybir
from concourse.bass_utils import run_bass_kernel_spmd

P = 128
B, T, E, VOC = 2, 2048, 1024, 32000
VSPLIT = 4                # vocab splits per batch group
VS = VOC // VSPLIT        # 8000 vocab columns per core
NE = E // P               # 8 embedding partition-tiles
NT = T // P               # 16 token partition-tiles
KL = NT // VSPLIT         # 4 local k-tiles per core (interleaved by dv)
TK = KL * P               # 512 key tokens per core
QCH = 512                 # q chunk width
NQC = T // QCH            # 4
VCH = 500                 # vocab chunk width (<=512 psum bank, 8000 = 16*500)
NVC = VS // VCH           # 16
SCALE = 1.0 / 32.0        # 1/sqrt(E)
MASK_VAL = -960000.0      # additive pre-scale mask; /32 -> -30000 -> exp = 0

BF16 = mybir.dt.bfloat16
F32 = mybir.dt.float32

# "rep3" (default): rep2 + fp8-DoubleRow k/q projections and score matmuls
#   + bf16 logits output (halves the dominant output-write DMA).
# "rep2": replicated attention per batch group, weight-amortized
#   loop order, all-bf16 (~338 us/core on HW).
# "rep": replicated attention, naive loop order (~476 us).
# "cc": k-sharded attention + AllReduce (~520 us; collective costs more
#   than the compute it saves at this size).
MODE = "rep3"

F8 = mybir.dt.float8e4
DR = mybir.MatmulPerfMode.DoubleRow
XS = 128.0       # x fp8 storage scale
WS = 256.0       # Wq/Wk fp8 storage scale
QS = 64.0        # q/k fp8 storage scale
S_PROJ = QS / (XS * WS)      # PSUM -> fp8 q/k copy scale
S_EXP = SCALE / (QS * QS)    # exp activation scale on fp8-scored PSUM
MASK_R3 = -1.0e9             # additive mask on raw scores (pre-activation)
OS = 128.0                   # attention-output fp8 storage scale (rep4)
ROW_S = 256.0                # Wro fp8 storage scale (rep4)
S_RO = 1.0 / (OS * ROW_S)    # readout PSUM -> bf16 logits scale (rep4)

_CACHE: dict = {}


def _emit_body_cc(tc, nc, aps, sfx):
    """k-sharded attention (interleaved) + AllReduce + vocab-sliced readout."""
    xT_d, xTk_d, wk_d, wq_d, wv_d, wro_d, mask_d, out_d = aps
    Exp = mybir.ActivationFunctionType.Exp
    groups = [[0, 1, 2, 3], [4, 5, 6, 7]]

    with ExitStack() as root:
        misc = root.enter_context(tc.tile_pool(name=f"misc{sfx}", bufs=1))
        psum = root.enter_context(tc.tile_pool(name=f"psum{sfx}", bufs=6, space="PSUM"))
        stage = root.enter_context(tc.tile_pool(name=f"stage{sfx}", bufs=2))
        dram = root.enter_context(tc.tile_pool(name=f"dram{sfx}", bufs=1, space="DRAM"))

        mask_t = misc.tile([P, QCH], F32, tag="mask", name=f"mask_t{sfx}")
        nc.sync.dma_start(mask_t[:], mask_d[:])
        parts_t = misc.tile([P, KL, NQC], F32, tag="parts", name=f"parts_t{sfx}")
        denom_t = misc.tile([P, KL], F32, tag="denom", name=f"denom_t{sfx}")
        recip_t = misc.tile([P, KL], F32, tag="recip", name=f"recip_t{sfx}")

        cc_in = dram.tile([NE, P, T], BF16, tag="cci", name=f"cc_in{sfx}")
        cc_out = dram.tile([NE, P, T], BF16, tag="cco", name=f"cc_out{sfx}")

        pkv = root.enter_context(tc.tile_pool(name=f"pkv{sfx}", bufs=1))
        kTl_t = [pkv.tile([P, TK], BF16, tag=f"kTl{i}", name=f"kTl{i}{sfx}") for i in range(NE)]
        vl_t = [pkv.tile([P, E], BF16, tag=f"vl{i}", name=f"vl{i}{sfx}") for i in range(KL)]

        pq = ExitStack()
        q_pool = pq.enter_context(tc.tile_pool(name=f"pq{sfx}", bufs=1))
        qT_t = [q_pool.tile([P, T], BF16, tag=f"qT{i}", name=f"qT{i}{sfx}") for i in range(NE)]

        p1 = ExitStack()
        x_pool = p1.enter_context(tc.tile_pool(name=f"px{sfx}", bufs=1))
        w_pool = p1.enter_context(tc.tile_pool(name=f"pw{sfx}", bufs=2))

        # ================= phase 1: projections =====================
        xT_t = [x_pool.tile([P, T], BF16, tag=f"xT{i}", name=f"xT{i}{sfx}") for i in range(NE)]
        for e in range(NE):
            nc.sync.dma_start(xT_t[e][:], xT_d[e])
        xTk_t = [x_pool.tile([P, TK], BF16, tag=f"xTk{i}", name=f"xTk{i}{sfx}") for i in range(NE)]
        for e in range(NE):
            nc.sync.dma_start(xTk_t[e][:], xTk_d[e])

        wq_t = w_pool.tile([P, NE, E], BF16, tag="w", name=f"wq_t{sfx}")
        for e in range(NE):
            nc.sync.dma_start(wq_t[:, e, :], wq_d[e])
        wk_t = w_pool.tile([P, NE, E], BF16, tag="w", name=f"wk_t{sfx}")
        for e in range(NE):
            nc.sync.dma_start(wk_t[:, e, :], wk_d[e])

        # qT[eo, t] = sum_e Wq[e, eo] * xT[e, t]   (full, replicated)
        for eo in range(NE):
            for tch in range(NQC):
                ps = psum.tile([P, QCH], F32, tag="mm", name=f"ps_q{eo}_{tch}{sfx}")
                for e in range(NE):
                    nc.tensor.matmul(
                        ps[:],
                        wq_t[:, e, eo * P:(eo + 1) * P],
                        xT_t[e][:, tch * QCH:(tch + 1) * QCH],
                        start=(e == 0), stop=(e == NE - 1),
                    )
                nc.scalar.copy(qT_t[eo][:, tch * QCH:(tch + 1) * QCH], ps[:])

        wv_t = w_pool.tile([P, NE, E], BF16, tag="w", name=f"wv_t{sfx}")
        for e in range(NE):
            nc.sync.dma_start(wv_t[:, e, :], wv_d[e])

        # kTl[eo, kloc] = sum_e Wk[e, eo] * xTk[e, kloc]  (local k slice)
        for eo in range(NE):
            ps = psum.tile([P, TK], F32, tag="mm", name=f"ps_k{eo}{sfx}")
            for e in range(NE):
                nc.tensor.matmul(
                    ps[:],
                    wk_t[:, e, eo * P:(eo + 1) * P],
                    xTk_t[e][:],
                    start=(e == 0), stop=(e == NE - 1),
                )
            nc.scalar.copy(kTl_t[eo][:], ps[:])

        # Vl[kloc, e] = sum_e' xTk[e', kloc]^T * Wv[e', e]
        for tb in range(KL):
            for ec in range(E // QCH):
                ps = psum.tile([P, QCH], F32, tag="mm", name=f"ps_v{tb}_{ec}{sfx}")
                for e in range(NE):
                    nc.tensor.matmul(
                        ps[:],
                        xTk_t[e][:, tb * P:(tb + 1) * P],
                        wv_t[:, e, ec * QCH:(ec + 1) * QCH],
                        start=(e == 0), stop=(e == NE - 1),
                    )
                nc.scalar.copy(vl_t[tb][:, ec * QCH:(ec + 1) * QCH], ps[:])

        p1.close()  # frees xT/xTk + W pools

        # ============ phase 2: scores + softmax-over-q ==============
        # local tile j == global k-tile dv+4j; diagonal chunk qc = j
        p2 = ExitStack()
        wT_pool = p2.enter_context(tc.tile_pool(name=f"pwT{sfx}", bufs=1, side="right"))
        wTl_t = [wT_pool.tile([P, T], BF16, tag=f"wTl{i}", name=f"wTl{i}{sfx}") for i in range(KL)]

        for j in range(KL):
            for qc in range(j, NQC):
                ps = psum.tile([P, QCH], F32, tag="mm", name=f"ps_s{j}_{qc}{sfx}")
                for e in range(NE):
                    nc.tensor.matmul(
                        ps[:],
                        kTl_t[e][:, j * P:(j + 1) * P],
                        qT_t[e][:, qc * QCH:(qc + 1) * QCH],
                        start=(e == 0), stop=(e == NE - 1),
                    )
                wslice = wTl_t[j][:, qc * QCH:(qc + 1) * QCH]
                acc = parts_t[:, j, qc:qc + 1]
                if qc == j:
                    stg = stage.tile([P, QCH], F32, tag="stg", name=f"stg{j}{sfx}")
                    nc.vector.tensor_add(stg[:], ps[:], mask_t[:])
                    nc.scalar.activation(wslice, stg[:], Exp, bias=0.0,
                                         scale=SCALE, accum_out=acc)
                else:
                    nc.scalar.activation(wslice, ps[:], Exp, bias=0.0,
                                         scale=SCALE, accum_out=acc)
            nc.vector.reduce_sum(denom_t[:, j:j + 1], parts_t[:, j, j:NQC],
                                 axis=mybir.AxisListType.X)
            nc.vector.reciprocal(recip_t[:, j:j + 1], denom_t[:, j:j + 1])
            nc.vector.tensor_scalar_mul(vl_t[j][:], vl_t[j][:], recip_t[:, j:j + 1])

        pq.close()  # frees qT

        # ====== phase 3: partial outT[e,q] -> AllReduce =============
        po_stage = ExitStack()
        av_pool = po_stage.enter_context(tc.tile_pool(name=f"pav{sfx}", bufs=4))
        for eb in range(NE):
            for qc in range(NQC):
                ps = psum.tile([P, QCH], F32, tag="mm", name=f"ps_o{eb}_{qc}{sfx}")
                njs = qc + 1  # local tiles j <= qc contribute
                for j in range(njs):
                    nc.tensor.matmul(
                        ps[:],
                        vl_t[j][:, eb * P:(eb + 1) * P],
                        wTl_t[j][:, qc * QCH:(qc + 1) * QCH],
                        start=(j == 0), stop=(j == njs - 1),
                    )
                stg = av_pool.tile([P, QCH], BF16, tag="av", name=f"av{eb}_{qc}{sfx}")
                nc.scalar.copy(stg[:], ps[:])
                nc.sync.dma_start(cc_in[eb, :, qc * QCH:(qc + 1) * QCH], stg[:])
        po_stage.close()
        p2.close()  # frees wTl

        nc.gpsimd.collective_compute(
            "AllReduce", mybir.AluOpType.add,
            replica_groups=groups,
            ins=[cc_in.opt()], outs=[cc_out.opt()],
        )

        # ============ phase 4: readout ==============================
        p3 = ExitStack()
        outT_pool = p3.enter_context(tc.tile_pool(name=f"poutT{sfx}", bufs=1))
        outT_t = [outT_pool.tile([P, T], BF16, tag=f"oT{i}", name=f"oT{i}{sfx}") for i in range(NE)]
        for e in range(NE):
            nc.sync.dma_start(outT_t[e][:], cc_out[e])

        p4 = ExitStack()
        ro_pool = p4.enter_context(tc.tile_pool(name=f"pro{sfx}", bufs=2))
        ostg_pool = p4.enter_context(tc.tile_pool(name=f"postg{sfx}", bufs=4))

        for vc in range(NVC):
            wro_t = ro_pool.tile([P, NE, VCH], BF16, tag="wro", name=f"wro{vc}{sfx}")
            for e in range(NE):
                nc.sync.dma_start(wro_t[:, e, :], wro_d[e, :, vc * VCH:(vc + 1) * VCH])
            for tb in range(NT):
                ps = psum.tile([P, VCH], F32, tag="mm", name=f"ps_r{vc}_{tb}{sfx}")
                for e in range(NE):
                    nc.tensor.matmul(
                        ps[:],
                        outT_t[e][:, tb * P:(tb + 1) * P],
                        wro_t[:, e, :],
                        start=(e == 0), stop=(e == NE - 1),
                    )
                stg = ostg_pool.tile([P, VCH], F32, tag="ostg", name=f"ostg{vc}_{tb}{sfx}")
                if tb % 2 == 0:
                    nc.vector.tensor_copy(stg[:], ps[:])
                else:
                    nc.scalar.copy(stg[:], ps[:])
                nc.sync.dma_start(out_d[tb, :, vc * VCH:(vc + 1) * VCH], stg[:])

        p4.close()
        p3.close()


def _emit_body_rep(tc, nc, aps, sfx):
    """Replicated attention per core (no collective) + vocab-sliced readout."""
    xT_d, xTk_d, wk_d, wq_d, wv_d, wro_d, mask4_d, out_d = aps
    Exp = mybir.ActivationFunctionType.Exp

    with ExitStack() as root:
        misc = root.enter_context(tc.tile_pool(name=f"misc{sfx}", bufs=1))
        psum = root.enter_context(tc.tile_pool(name=f"psum{sfx}", bufs=6, space="PSUM"))
        stage = root.enter_context(tc.tile_pool(name=f"stage{sfx}", bufs=2))

        mask_t = misc.tile([P, NQC, QCH], F32, tag="mask", name=f"mask_t{sfx}")
        nc.sync.dma_start(mask_t[:], mask4_d[:])
        parts_t = misc.tile([P, NT, NQC], F32, tag="parts", name=f"parts_t{sfx}")
        denom_t = misc.tile([P, NT], F32, tag="denom", name=f"denom_t{sfx}")
        recip_t = misc.tile([P, NT], F32, tag="recip", name=f"recip_t{sfx}")

        pv = root.enter_context(tc.tile_pool(name=f"pv{sfx}", bufs=1))
        v_t = [pv.tile([P, E], BF16, tag=f"v{i}", name=f"v{i}{sfx}") for i in range(NT)]

        pkq = ExitStack()
        kq_pool = pkq.enter_context(tc.tile_pool(name=f"pkq{sfx}", bufs=1))
        kT_t = [kq_pool.tile([P, T], BF16, tag=f"kT{i}", name=f"kT{i}{sfx}") for i in range(NE)]
        qT_t = [kq_pool.tile([P, T], BF16, tag=f"qT{i}", name=f"qT{i}{sfx}") for i in range(NE)]

        p1 = ExitStack()
        x_pool = p1.enter_context(tc.tile_pool(name=f"px{sfx}", bufs=1))
        w_pool = p1.enter_context(tc.tile_pool(name=f"pw{sfx}", bufs=2))

        xT_t = [x_pool.tile([P, T], BF16, tag=f"xT{i}", name=f"xT{i}{sfx}") for i in range(NE)]
        for e in range(NE):
            nc.sync.dma_start(xT_t[e][:], xT_d[e])

        wk_t = w_pool.tile([P, NE, E], BF16, tag="w", name=f"wk_t{sfx}")
        for e in range(NE):
            nc.sync.dma_start(wk_t[:, e, :], wk_d[e])
        wq_t = w_pool.tile([P, NE, E], BF16, tag="w", name=f"wq_t{sfx}")
        for e in range(NE):
            nc.sync.dma_start(wq_t[:, e, :], wq_d[e])

        for eo in range(NE):
            for tch in range(NQC):
                ps = psum.tile([P, QCH], F32, tag="mm", name=f"ps_k{eo}_{tch}{sfx}")
                for e in range(NE):
                    nc.tensor.matmul(
                        ps[:],
                        wk_t[:, e, eo * P:(eo + 1) * P],
                        xT_t[e][:, tch * QCH:(tch + 1) * QCH],
                        start=(e == 0), stop=(e == NE - 1),
                    )
                nc.scalar.copy(kT_t[eo][:, tch * QCH:(tch + 1) * QCH], ps[:])

        wv_t = w_pool.tile([P, NE, E], BF16, tag="w", name=f"wv_t{sfx}")
        for e in range(NE):
            nc.sync.dma_start(wv_t[:, e, :], wv_d[e])

        for eo in range(NE):
            for tch in range(NQC):
                ps = psum.tile([P, QCH], F32, tag="mm", name=f"ps_q{eo}_{tch}{sfx}")
                for e in range(NE):
                    nc.tensor.matmul(
                        ps[:],
                        wq_t[:, e, eo * P:(eo + 1) * P],
                        xT_t[e][:, tch * QCH:(tch + 1) * QCH],
                        start=(e == 0), stop=(e == NE - 1),
                    )
                nc.scalar.copy(qT_t[eo][:, tch * QCH:(tch + 1) * QCH], ps[:])

        for tb in range(NT):
            for ec in range(E // QCH):
                ps = psum.tile([P, QCH], F32, tag="mm", name=f"ps_v{tb}_{ec}{sfx}")
                for e in range(NE):
                    nc.tensor.matmul(
                        ps[:],
                        xT_t[e][:, tb * P:(tb + 1) * P],
                        wv_t[:, e, ec * QCH:(ec + 1) * QCH],
                        start=(e == 0), stop=(e == NE - 1),
                    )
                nc.scalar.copy(v_t[tb][:, ec * QCH:(ec + 1) * QCH], ps[:])

        p1.close()

        p2 = ExitStack()
        wT_pool = p2.enter_context(tc.tile_pool(name=f"pwT{sfx}", bufs=1, side="right"))
        wT_t = [wT_pool.tile([P, T], BF16, tag=f"wT{i}", name=f"wT{i}{sfx}") for i in range(NT)]

        for kt in range(NT):
            qcd = kt // 4
            for qc in range(qcd, NQC):
                ps = psum.tile([P, QCH], F32, tag="mm", name=f"ps_s{kt}_{qc}{sfx}")
                for e in range(NE):
                    nc.tensor.matmul(
                        ps[:],
                        kT_t[e][:, kt * P:(kt + 1) * P],
                        qT_t[e][:, qc * QCH:(qc + 1) * QCH],
                        start=(e == 0), stop=(e == NE - 1),
                    )
                wslice = wT_t[kt][:, qc * QCH:(qc + 1) * QCH]
                acc = parts_t[:, kt, qc:qc + 1]
                if qc == qcd:
                    dv = kt % 4
                    stg = stage.tile([P, QCH], F32, tag="stg", name=f"stg{kt}{sfx}")
                    nc.vector.tensor_add(stg[:], ps[:], mask_t[:, dv, :])
                    nc.scalar.activation(wslice, stg[:], Exp, bias=0.0,
                                         scale=SCALE, accum_out=acc)
                else:
                    nc.scalar.activation(wslice, ps[:], Exp, bias=0.0,
                                         scale=SCALE, accum_out=acc)
            nc.vector.reduce_sum(denom_t[:, kt:kt + 1], parts_t[:, kt, qcd:NQC],
                                 axis=mybir.AxisListType.X)
            nc.vector.reciprocal(recip_t[:, kt:kt + 1], denom_t[:, kt:kt + 1])
            nc.vector.tensor_scalar_mul(v_t[kt][:], v_t[kt][:], recip_t[:, kt:kt + 1])

        pkq.close()

        p3 = ExitStack()
        outT_pool = p3.enter_context(tc.tile_pool(name=f"poutT{sfx}", bufs=1))
        outT_t = [outT_pool.tile([P, T], BF16, tag=f"oT{i}", name=f"oT{i}{sfx}") for i in range(NE)]

        for eb in range(NE):
            for qc in range(NQC):
                kts = list(range(0, (qc + 1) * 4))
                ps = psum.tile([P, QCH], F32, tag="mm", name=f"ps_o{eb}_{qc}{sfx}")
                for i, kt in enumerate(kts):
                    nc.tensor.matmul(
                        ps[:],
                        v_t[kt][:, eb * P:(eb + 1) * P],
                        wT_t[kt][:, qc * QCH:(qc + 1) * QCH],
                        start=(i == 0), stop=(i == len(kts) - 1),
                    )
                nc.scalar.copy(outT_t[eb][:, qc * QCH:(qc + 1) * QCH], ps[:])

        p2.close()

        p4 = ExitStack()
        ro_pool = p4.enter_context(tc.tile_pool(name=f"pro{sfx}", bufs=2))
        ostg_pool = p4.enter_context(tc.tile_pool(name=f"postg{sfx}", bufs=4))

        for vc in range(NVC):
            wro_t = ro_pool.tile([P, NE, VCH], BF16, tag="wro", name=f"wro{vc}{sfx}")
            for e in range(NE):
                nc.sync.dma_start(wro_t[:, e, :], wro_d[e, :, vc * VCH:(vc + 1) * VCH])
            for tb in range(NT):
                ps = psum.tile([P, VCH], F32, tag="mm", name=f"ps_r{vc}_{tb}{sfx}")
                for e in range(NE):
                    nc.tensor.matmul(
                        ps[:],
                        outT_t[e][:, tb * P:(tb + 1) * P],
                        wro_t[:, e, :],
                        start=(e == 0), stop=(e == NE - 1),
                    )
                stg = ostg_pool.tile([P, VCH], F32, tag="ostg", name=f"ostg{vc}_{tb}{sfx}")
                if tb % 2 == 0:
                    nc.vector.tensor_copy(stg[:], ps[:])
                else:
                    nc.scalar.copy(stg[:], ps[:])
                nc.sync.dma_start(out_d[tb, :, vc * VCH:(vc + 1) * VCH], stg[:])

        p4.close()
        p3.close()


def _emit_body_rep2(tc, nc, aps, sfx):
    """Replicated attention, weight-amortized loop order: each stationary
    operand (lhsT) is reused across several matmuls feeding parallel PSUM
    banks, so LDWEIGHTS traffic drops ~3-4x."""
    xT_d, xTk_d, wk_d, wq_d, wv_d, wro_d, mask4_d, out_d = aps
    Exp = mybir.ActivationFunctionType.Exp

    with ExitStack() as root:
        misc = root.enter_context(tc.tile_pool(name=f"misc{sfx}", bufs=1))
        psum = root.enter_context(tc.tile_pool(name=f"psum{sfx}", bufs=8, space="PSUM"))
        stage = root.enter_context(tc.tile_pool(name=f"stage{sfx}", bufs=2))

        mask_t = misc.tile([P, NQC, QCH], F32, tag="mask", name=f"mask_t{sfx}")
        nc.sync.dma_start(mask_t[:], mask4_d[:])
        parts_t = misc.tile([P, NT, NQC], F32, tag="parts", name=f"parts_t{sfx}")
        denom_t = misc.tile([P, NT], F32, tag="denom", name=f"denom_t{sfx}")
        recip_t = misc.tile([P, NT], F32, tag="recip", name=f"recip_t{sfx}")

        pv = root.enter_context(tc.tile_pool(name=f"pv{sfx}", bufs=1))
        v_t = [pv.tile([P, E], BF16, tag=f"v{i}", name=f"v{i}{sfx}") for i in range(NT)]

        pkq = ExitStack()
        kq_pool = pkq.enter_context(tc.tile_pool(name=f"pkq{sfx}", bufs=1))
        kT_t = [kq_pool.tile([P, T], BF16, tag=f"kT{i}", name=f"kT{i}{sfx}") for i in range(NE)]
        qT_t = [kq_pool.tile([P, T], BF16, tag=f"qT{i}", name=f"qT{i}{sfx}") for i in range(NE)]

        p1 = ExitStack()
        x_pool = p1.enter_context(tc.tile_pool(name=f"px{sfx}", bufs=1))
        w_pool = p1.enter_context(tc.tile_pool(name=f"pw{sfx}", bufs=2))

        xT_t = [x_pool.tile([P, T], BF16, tag=f"xT{i}", name=f"xT{i}{sfx}") for i in range(NE)]
        for e in range(NE):
            nc.sync.dma_start(xT_t[e][:], xT_d[e])

        wk_t = w_pool.tile([P, NE, E], BF16, tag="w", name=f"wk_t{sfx}")
        for e in range(NE):
            nc.sync.dma_start(wk_t[:, e, :], wk_d[e])
        wq_t = w_pool.tile([P, NE, E], BF16, tag="w", name=f"wq_t{sfx}")
        for e in range(NE):
            nc.sync.dma_start(wq_t[:, e, :], wq_d[e])

        # kT: weight wk[:,e,eo] loaded once per (eo,e), reused for 4 t-chunks
        for eo in range(NE):
            psA = [psum.tile([P, QCH], F32, tag="mm", name=f"ps_k{eo}_{t_}{sfx}")
                   for t_ in range(NQC)]
            for e in range(NE):
                for tch in range(NQC):
                    nc.tensor.matmul(
                        psA[tch][:],
                        wk_t[:, e, eo * P:(eo + 1) * P],
                        xT_t[e][:, tch * QCH:(tch + 1) * QCH],
                        start=(e == 0), stop=(e == NE - 1),
                    )
            for tch in range(NQC):
                nc.scalar.copy(kT_t[eo][:, tch * QCH:(tch + 1) * QCH], psA[tch][:])

        wv_t = w_pool.tile([P, NE, E], BF16, tag="w", name=f"wv_t{sfx}")
        for e in range(NE):
            nc.sync.dma_start(wv_t[:, e, :], wv_d[e])

        for eo in range(NE):
            psA = [psum.tile([P, QCH], F32, tag="mm", name=f"ps_q{eo}_{t_}{sfx}")
                   for t_ in range(NQC)]
            for e in range(NE):
                for tch in range(NQC):
                    nc.tensor.matmul(
                        psA[tch][:],
                        wq_t[:, e, eo * P:(eo + 1) * P],
                        xT_t[e][:, tch * QCH:(tch + 1) * QCH],
                        start=(e == 0), stop=(e == NE - 1),
                    )
            for tch in range(NQC):
                nc.scalar.copy(qT_t[eo][:, tch * QCH:(tch + 1) * QCH], psA[tch][:])

        # V: weight xT[e][:,tb] loaded once per (tb,e), reused for 2 e-chunks
        for tb in range(NT):
            psA = [psum.tile([P, QCH], F32, tag="mm", name=f"ps_v{tb}_{ec}{sfx}")
                   for ec in range(E // QCH)]
            for e in range(NE):
                for ec in range(E // QCH):
                    nc.tensor.matmul(
                        psA[ec][:],
                        xT_t[e][:, tb * P:(tb + 1) * P],
                        wv_t[:, e, ec * QCH:(ec + 1) * QCH],
                        start=(e == 0), stop=(e == NE - 1),
                    )
            for ec in range(E // QCH):
                nc.scalar.copy(v_t[tb][:, ec * QCH:(ec + 1) * QCH], psA[ec][:])

        p1.close()

        p2 = ExitStack()
        wT_pool = p2.enter_context(tc.tile_pool(name=f"pwT{sfx}", bufs=1, side="right"))
        wT_t = [wT_pool.tile([P, T], BF16, tag=f"wT{i}", name=f"wT{i}{sfx}") for i in range(NT)]

        # scores: weight kT[e][:,kt] loaded once per (kt,e), reused for the
        # computed q-chunks
        for kt in range(NT):
            qcd = kt // 4
            psA = {qc: psum.tile([P, QCH], F32, tag="mm", name=f"ps_s{kt}_{qc}{sfx}")
                   for qc in range(qcd, NQC)}
            for e in range(NE):
                for qc in range(qcd, NQC):
                    nc.tensor.matmul(
                        psA[qc][:],
                        kT_t[e][:, kt * P:(kt + 1) * P],
                        qT_t[e][:, qc * QCH:(qc + 1) * QCH],
                        start=(e == 0), stop=(e == NE - 1),
                    )
            for qc in range(qcd, NQC):
                wslice = wT_t[kt][:, qc * QCH:(qc + 1) * QCH]
                acc = parts_t[:, kt, qc:qc + 1]
                if qc == qcd:
                    dv = kt % 4
                    stg = stage.tile([P, QCH], F32, tag="stg", name=f"stg{kt}{sfx}")
                    nc.vector.tensor_add(stg[:], psA[qc][:], mask_t[:, dv, :])
                    nc.scalar.activation(wslice, stg[:], Exp, bias=0.0,
                                         scale=SCALE, accum_out=acc)
                else:
                    nc.scalar.activation(wslice, psA[qc][:], Exp, bias=0.0,
                                         scale=SCALE, accum_out=acc)
            nc.vector.reduce_sum(denom_t[:, kt:kt + 1], parts_t[:, kt, qcd:NQC],
                                 axis=mybir.AxisListType.X)
            nc.vector.reciprocal(recip_t[:, kt:kt + 1], denom_t[:, kt:kt + 1])
            nc.vector.tensor_scalar_mul(v_t[kt][:], v_t[kt][:], recip_t[:, kt:kt + 1])

        pkq.close()

        p3 = ExitStack()
        outT_pool = p3.enter_context(tc.tile_pool(name=f"poutT{sfx}", bufs=1))
        outT_t = [outT_pool.tile([P, T], BF16, tag=f"oT{i}", name=f"oT{i}{sfx}") for i in range(NE)]

        # AV: weight v_t[kt][:,eb] loaded once per (eb,kt), reused for the
        # q-chunks that include kt
        for eb in range(NE):
            psA = [psum.tile([P, QCH], F32, tag="mm", name=f"ps_o{eb}_{qc}{sfx}")
                   for qc in range(NQC)]
            for kt in range(NT):
                for qc in range(NQC):
                    if kt >= (qc + 1) * 4:
                        continue
                    nc.tensor.matmul(
                        psA[qc][:],
                        v_t[kt][:, eb * P:(eb + 1) * P],
                        wT_t[kt][:, qc * QCH:(qc + 1) * QCH],
                        start=(kt == 0), stop=(kt == (qc + 1) * 4 - 1),
                    )
            for qc in range(NQC):
                nc.scalar.copy(outT_t[eb][:, qc * QCH:(qc + 1) * QCH], psA[qc][:])

        p2.close()

        p4 = ExitStack()
        ro_pool = p4.enter_context(tc.tile_pool(name=f"pro{sfx}", bufs=2))
        ostg_pool = p4.enter_context(tc.tile_pool(name=f"postg{sfx}", bufs=4))

        # readout: weight outT[e][:,tb] loaded once per (tb,e), reused for a
        # group of 4 vocab chunks
        VGRP = 4
        for vg in range(NVC // VGRP):
            wro_t = ro_pool.tile([P, NE, VGRP, VCH], BF16, tag="wro", name=f"wro{vg}{sfx}")
            for e in range(NE):
                for vq in range(VGRP):
                    vc = vg * VGRP + vq
                    nc.sync.dma_start(wro_t[:, e, vq, :],
                                      wro_d[e, :, vc * VCH:(vc + 1) * VCH])
            for tb in range(NT):
                psA = [psum.tile([P, VCH], F32, tag="mm", name=f"ps_r{vg}_{tb}_{vq}{sfx}")
                       for vq in range(VGRP)]
                for e in range(NE):
                    for vq in range(VGRP):
                        nc.tensor.matmul(
                            psA[vq][:],
                            outT_t[e][:, tb * P:(tb + 1) * P],
                            wro_t[:, e, vq, :],
                            start=(e == 0), stop=(e == NE - 1),
                        )
                for vq in range(VGRP):
                    vc = vg * VGRP + vq
                    stg = ostg_pool.tile([P, VCH], F32, tag="ostg",
                                         name=f"ostg{vc}_{tb}{sfx}")
                    if vq % 2 == 0:
                        nc.vector.tensor_copy(stg[:], psA[vq][:])
                    else:
                        nc.scalar.copy(stg[:], psA[vq][:])
                    nc.sync.dma_start(out_d[tb, :, vc * VCH:(vc + 1) * VCH], stg[:])

        p4.close()
        p3.close()


def _emit_body_rep3(tc, nc, aps, sfx):
    """rep2 + fp8 DoubleRow for k/q projections and score matmuls (PE halved
    on those phases) + bf16 logits staging (halves output DMA).  v / AV /
    readout stay bf16 — fp8 there would push rel_err past the 2e-2 gate."""
    xT_d, xT8_d, wk8_d, wq8_d, wvp_d, wro_d, mask4_d, out_d = aps
    Exp = mybir.ActivationFunctionType.Exp
    Copy = mybir.ActivationFunctionType.Copy
    NEP = NE // 2  # fp8 DoubleRow pairs along the contraction dim

    with ExitStack() as root:
        misc = root.enter_context(tc.tile_pool(name=f"misc{sfx}", bufs=1))
        psum = root.enter_context(tc.tile_pool(name=f"psum{sfx}", bufs=8, space="PSUM"))
        stage = root.enter_context(tc.tile_pool(name=f"stage{sfx}", bufs=2))

        mask_t = misc.tile([P, NQC, QCH], F32, tag="mask", name=f"mask_t{sfx}")
        nc.sync.dma_start(mask_t[:], mask4_d[:])
        parts_t = misc.tile([P, NT, NQC], F32, tag="parts", name=f"parts_t{sfx}")
        denom_t = misc.tile([P, NT], F32, tag="denom", name=f"denom_t{sfx}")
        recip_t = misc.tile([P, NT], F32, tag="recip", name=f"recip_t{sfx}")

        pv = root.enter_context(tc.tile_pool(name=f"pv{sfx}", bufs=1))
        v_t = [pv.tile([P, E], BF16, tag=f"v{i}", name=f"v{i}{sfx}") for i in range(NT)]

        pkq = ExitStack()
        kq_pool = pkq.enter_context(tc.tile_pool(name=f"pkq{sfx}", bufs=1))
        kT8_t = kq_pool.tile([P, NE, T], F8, tag="kT8", name=f"kT8{sfx}")
        qT8_t = kq_pool.tile([P, NE, T], F8, tag="qT8", name=f"qT8{sfx}")

        p1 = ExitStack()
        x_pool = p1.enter_context(tc.tile_pool(name=f"px{sfx}", bufs=1))
        w_pool = p1.enter_context(tc.tile_pool(name=f"pw{sfx}", bufs=1))

        xT_t = [x_pool.tile([P, T], BF16, tag=f"xT{i}", name=f"xT{i}{sfx}") for i in range(NE)]
        for e in range(NE):
            nc.sync.dma_start(xT_t[e][:], xT_d[e])
        xT8_t = x_pool.tile([P, NE, T], F8, tag="xT8", name=f"xT8{sfx}")
        nc.sync.dma_start(xT8_t[:], xT8_d[:])

        wk8_t = w_pool.tile([P, NE, E], F8, tag="wk8", name=f"wk8_t{sfx}")
        nc.sync.dma_start(wk8_t[:], wk8_d[:])
        wq8_t = w_pool.tile([P, NE, E], F8, tag="wq8", name=f"wq8_t{sfx}")
        nc.sync.dma_start(wq8_t[:], wq8_d[:])
        wv_t = w_pool.tile([P, NE, E], BF16, tag="wv", name=f"wv_t{sfx}")
        nc.sync.dma_start(wv_t[:], wvp_d[:])

        # k/q projections, fp8 DoubleRow: stationary w-pair reused over 4
        # t-chunks; psum = x@W * (XS*WS), stored to fp8 at QS.
        for (w8_t, dst) in ((wk8_t, kT8_t), (wq8_t, qT8_t)):
            for eo in range(NE):
                psA = [psum.tile([P, QCH], F32, tag="mm", name=f"ps_{eo}_{t_}{sfx}")
                       for t_ in range(NQC)]
                for ep in range(NEP):
                    for tch in range(NQC):
                        nc.tensor.matmul(
                            psA[tch][:],
                            w8_t[:, 2 * ep:2 * ep + 2, eo * P:(eo + 1) * P],
                            xT8_t[:, 2 * ep:2 * ep + 2, tch * QCH:(tch + 1) * QCH],
                            start=(ep == 0), stop=(ep == NEP - 1),
                            perf_mode=DR,
                        )
                for tch in range(NQC):
                    nc.scalar.activation(
                        dst[:, eo, tch * QCH:(tch + 1) * QCH], psA[tch][:],
                        Copy, bias=0.0, scale=S_PROJ)

        # V projection (bf16): stationary xT[:, tb] reused over 2 e-chunks
        for tb in range(NT):
            psA = [psum.tile([P, QCH], F32, tag="mm", name=f"ps_v{tb}_{ec}{sfx}")
                   for ec in range(E // QCH)]
            for e in range(NE):
                for ec in range(E // QCH):
                    nc.tensor.matmul(
                        psA[ec][:],
                        xT_t[e][:, tb * P:(tb + 1) * P],
                        wv_t[:, e, ec * QCH:(ec + 1) * QCH],
                        start=(e == 0), stop=(e == NE - 1),
                    )
            for ec in range(E // QCH):
                nc.scalar.copy(v_t[tb][:, ec * QCH:(ec + 1) * QCH], psA[ec][:])

        p1.close()

        p2 = ExitStack()
        wT_pool = p2.enter_context(tc.tile_pool(name=f"pwT{sfx}", bufs=1, side="right"))
        wT_t = [wT_pool.tile([P, T], BF16, tag=f"wT{i}", name=f"wT{i}{sfx}") for i in range(NT)]

        # scores, fp8 DoubleRow: psum = q.k * QS^2; exp scale folds it back
        for kt in range(NT):
            qcd = kt // 4
            psA = {qc: psum.tile([P, QCH], F32, tag="mm", name=f"ps_s{kt}_{qc}{sfx}")
                   for qc in range(qcd, NQC)}
            for ep in range(NEP):
                for qc in range(qcd, NQC):
                    nc.tensor.matmul(
                        psA[qc][:],
                        kT8_t[:, 2 * ep:2 * ep + 2, kt * P:(kt + 1) * P],
                        qT8_t[:, 2 * ep:2 * ep + 2, qc * QCH:(qc + 1) * QCH],
                        start=(ep == 0), stop=(ep == NEP - 1),
                        perf_mode=DR,
                    )
            for qc in range(qcd, NQC):
                wslice = wT_t[kt][:, qc * QCH:(qc + 1) * QCH]
                acc = parts_t[:, kt, qc:qc + 1]
                if qc == qcd:
                    dv = kt % 4
                    stg = stage.tile([P, QCH], F32, tag="stg", name=f"stg{kt}{sfx}")
                    nc.vector.tensor_add(stg[:], psA[qc][:], mask_t[:, dv, :])
                    nc.scalar.activation(wslice, stg[:], Exp, bias=0.0,
                                         scale=S_EXP, accum_out=acc)
                else:
                    nc.scalar.activation(wslice, psA[qc][:], Exp, bias=0.0,
                                         scale=S_EXP, accum_out=acc)
            nc.vector.reduce_sum(denom_t[:, kt:kt + 1], parts_t[:, kt, qcd:NQC],
                                 axis=mybir.AxisListType.X)
            nc.vector.reciprocal(recip_t[:, kt:kt + 1], denom_t[:, kt:kt + 1])
            nc.vector.tensor_scalar_mul(v_t[kt][:], v_t[kt][:], recip_t[:, kt:kt + 1])

        pkq.close()

        p3 = ExitStack()
        outT_pool = p3.enter_context(tc.tile_pool(name=f"poutT{sfx}", bufs=1))
        outT_t = [outT_pool.tile([P, T], BF16, tag=f"oT{i}", name=f"oT{i}{sfx}") for i in range(NE)]

        # AV (bf16): stationary v_t[kt][:, eb] reused over valid q-chunks
        for eb in range(NE):
            psA = [psum.tile([P, QCH], F32, tag="mm", name=f"ps_o{eb}_{qc}{sfx}")
                   for qc in range(NQC)]
            for kt in range(NT):
                for qc in range(NQC):
                    if kt >= (qc + 1) * 4:
                        continue
                    nc.tensor.matmul(
                        psA[qc][:],
                        v_t[kt][:, eb * P:(eb + 1) * P],
                        wT_t[kt][:, qc * QCH:(qc + 1) * QCH],
                        start=(kt == 0), stop=(kt == (qc + 1) * 4 - 1),
                    )
            for qc in range(NQC):
                nc.scalar.copy(outT_t[eb][:, qc * QCH:(qc + 1) * QCH], psA[qc][:])

        p2.close()

        p4 = ExitStack()
        ro_pool = p4.enter_context(tc.tile_pool(name=f"pro{sfx}", bufs=2))
        ostg_pool = p4.enter_context(tc.tile_pool(name=f"postg{sfx}", bufs=4))

        # readout (bf16): stationary outT[e][:, tb] reused over 4 vocab chunks
        VGRP = 4
        for vg in range(NVC // VGRP):
            wro_t = ro_pool.tile([P, NE, VGRP, VCH], BF16, tag="wro", name=f"wro{vg}{sfx}")
            for e in range(NE):
                for vq in range(VGRP):
                    vc = vg * VGRP + vq
                    nc.sync.dma_start(wro_t[:, e, vq, :],
                                      wro_d[e, :, vc * VCH:(vc + 1) * VCH])
            for tb in range(NT):
                psA = [psum.tile([P, VCH], F32, tag="mm", name=f"ps_r{vg}_{tb}_{vq}{sfx}")
                       for vq in range(VGRP)]
                for e in range(NE):
                    for vq in range(VGRP):
                        nc.tensor.matmul(
                            psA[vq][:],
                            outT_t[e][:, tb * P:(tb + 1) * P],
                            wro_t[:, e, vq, :],
                            start=(e == 0), stop=(e == NE - 1),
                        )
                for vq in range(VGRP):
                    vc = vg * VGRP + vq
                    stg = ostg_pool.tile([P, VCH], BF16, tag="ostg",
                                         name=f"ostg{vc}_{tb}{sfx}")
                    if vq % 2 == 0:
                        nc.vector.tensor_copy(stg[:], psA[vq][:])
                    else:
                        nc.scalar.copy(stg[:], psA[vq][:])
                    nc.sync.dma_start(out_d[tb, :, vc * VCH:(vc + 1) * VCH], stg[:])

        p4.close()
        p3.close()


def _emit_body_rep4(tc, nc, aps, sfx):
    """rep3 + error-compensated fp8 readout.

    The attention output o and Wro are each split into fp8 (high, low)
    parts at a shared scale: a ~= a_h + a_l with a_l the rounding residual
    (stored directly in fp8 — residuals land in the normal/subnormal range
    where their own rounding error is ~0.4% of a, i.e. negligible).  Then

        logits ~= o_h@W_h + o_h@W_l + o_l@W_h      (o_l@W_l dropped)

    and all three terms share one PSUM accumulation at the same scale, so
    the epilogue is the same single scaled copy as bf16.  12 DoubleRow
    matmuls replace 8 bf16 matmuls per output tile: 25% fewer PE cycles
    on the dominant phase at full bf16-level accuracy."""
    xT_d, xT8_d, wk8_d, wq8_d, wvp_d, wro8h_d, wro8l_d, mask4_d, out_d = aps
    Exp = mybir.ActivationFunctionType.Exp
    Copy = mybir.ActivationFunctionType.Copy
    NEP = NE // 2

    with ExitStack() as root:
        misc = root.enter_context(tc.tile_pool(name=f"misc{sfx}", bufs=1))
        psum = root.enter_context(tc.tile_pool(name=f"psum{sfx}", bufs=8, space="PSUM"))
        stage = root.enter_context(tc.tile_pool(name=f"stage{sfx}", bufs=2))

        mask_t = misc.tile([P, NQC, QCH], F32, tag="mask", name=f"mask_t{sfx}")
        nc.sync.dma_start(mask_t[:], mask4_d[:])
        parts_t = misc.tile([P, NT, NQC], F32, tag="parts", name=f"parts_t{sfx}")
        denom_t = misc.tile([P, NT], F32, tag="denom", name=f"denom_t{sfx}")
        recip_t = misc.tile([P, NT], F32, tag="recip", name=f"recip_t{sfx}")
        recs_t = misc.tile([P, NT], F32, tag="recs", name=f"recs_t{sfx}")

        pv = root.enter_context(tc.tile_pool(name=f"pv{sfx}", bufs=1))
        v_t = [pv.tile([P, E], BF16, tag=f"v{i}", name=f"v{i}{sfx}") for i in range(NT)]

        pkq = ExitStack()
        kq_pool = pkq.enter_context(tc.tile_pool(name=f"pkq{sfx}", bufs=1))
        kT8_t = kq_pool.tile([P, NE, T], F8, tag="kT8", name=f"kT8{sfx}")
        qT8_t = kq_pool.tile([P, NE, T], F8, tag="qT8", name=f"qT8{sfx}")

        p1 = ExitStack()
        x_pool = p1.enter_context(tc.tile_pool(name=f"px{sfx}", bufs=1))
        w_pool = p1.enter_context(tc.tile_pool(name=f"pw{sfx}", bufs=1))

        xT_t = [x_pool.tile([P, T], BF16, tag=f"xT{i}", name=f"xT{i}{sfx}") for i in range(NE)]
        for e in range(NE):
            nc.sync.dma_start(xT_t[e][:], xT_d[e])
        xT8_t = x_pool.tile([P, NE, T], F8, tag="xT8", name=f"xT8{sfx}")
        nc.sync.dma_start(xT8_t[:], xT8_d[:])

        wk8_t = w_pool.tile([P, NE, E], F8, tag="wk8", name=f"wk8_t{sfx}")
        nc.sync.dma_start(wk8_t[:], wk8_d[:])
        wq8_t = w_pool.tile([P, NE, E], F8, tag="wq8", name=f"wq8_t{sfx}")
        nc.sync.dma_start(wq8_t[:], wq8_d[:])
        wv_t = w_pool.tile([P, NE, E], BF16, tag="wv", name=f"wv_t{sfx}")
        nc.sync.dma_start(wv_t[:], wvp_d[:])

        for (w8_t, dst) in ((wk8_t, kT8_t), (wq8_t, qT8_t)):
            for eo in range(NE):
                psA = [psum.tile([P, QCH], F32, tag="mm", name=f"ps_{eo}_{t_}{sfx}")
                       for t_ in range(NQC)]
                for ep in range(NEP):
                    for tch in range(NQC):
                        nc.tensor.matmul(
                            psA[tch][:],
                            w8_t[:, 2 * ep:2 * ep + 2, eo * P:(eo + 1) * P],
                            xT8_t[:, 2 * ep:2 * ep + 2, tch * QCH:(tch + 1) * QCH],
                            start=(ep == 0), stop=(ep == NEP - 1),
                            perf_mode=DR,
                        )
                for tch in range(NQC):
                    nc.scalar.activation(
                        dst[:, eo, tch * QCH:(tch + 1) * QCH], psA[tch][:],
                        Copy, bias=0.0, scale=S_PROJ)

        for tb in range(NT):
            psA = [psum.tile([P, QCH], F32, tag="mm", name=f"ps_v{tb}_{ec}{sfx}")
                   for ec in range(E // QCH)]
            for e in range(NE):
                for ec in range(E // QCH):
                    nc.tensor.matmul(
                        psA[ec][:],
                        xT_t[e][:, tb * P:(tb + 1) * P],
                        wv_t[:, e, ec * QCH:(ec + 1) * QCH],
                        start=(e == 0), stop=(e == NE - 1),
                    )
            for ec in range(E // QCH):
                nc.scalar.copy(v_t[tb][:, ec * QCH:(ec + 1) * QCH], psA[ec][:])

        p1.close()

        p2 = ExitStack()
        wT_pool = p2.enter_context(tc.tile_pool(name=f"pwT{sfx}", bufs=1, side="right"))
        wT_t = [wT_pool.tile([P, T], BF16, tag=f"wT{i}", name=f"wT{i}{sfx}") for i in range(NT)]

        for kt in range(NT):
            qcd = kt // 4
            psA = {qc: psum.tile([P, QCH], F32, tag="mm", name=f"ps_s{kt}_{qc}{sfx}")
                   for qc in range(qcd, NQC)}
            for ep in range(NEP):
                for qc in range(qcd, NQC):
                    nc.tensor.matmul(
                        psA[qc][:],
                        kT8_t[:, 2 * ep:2 * ep + 2, kt * P:(kt + 1) * P],
                        qT8_t[:, 2 * ep:2 * ep + 2, qc * QCH:(qc + 1) * QCH],
                        start=(ep == 0), stop=(ep == NEP - 1),
                        perf_mode=DR,
                    )
            for qc in range(qcd, NQC):
                wslice = wT_t[kt][:, qc * QCH:(qc + 1) * QCH]
                acc = parts_t[:, kt, qc:qc + 1]
                if qc == qcd:
                    dv = kt % 4
                    stg = stage.tile([P, QCH], F32, tag="stg", name=f"stg{kt}{sfx}")
                    nc.vector.tensor_add(stg[:], psA[qc][:], mask_t[:, dv, :])
                    nc.scalar.activation(wslice, stg[:], Exp, bias=0.0,
                                         scale=S_EXP, accum_out=acc)
                else:
                    nc.scalar.activation(wslice, psA[qc][:], Exp, bias=0.0,
                                         scale=S_EXP, accum_out=acc)
            nc.vector.reduce_sum(denom_t[:, kt:kt + 1], parts_t[:, kt, qcd:NQC],
                                 axis=mybir.AxisListType.X)
            nc.vector.reciprocal(recip_t[:, kt:kt + 1], denom_t[:, kt:kt + 1])
            # fold the fp8 storage scale OS for o into the softmax denominator
            nc.scalar.activation(recs_t[:, kt:kt + 1], recip_t[:, kt:kt + 1],
                                 Copy, bias=0.0, scale=OS)
            nc.vector.tensor_scalar_mul(v_t[kt][:], v_t[kt][:], recs_t[:, kt:kt + 1])

        pkq.close()

        p3 = ExitStack()
        outT_pool = p3.enter_context(tc.tile_pool(name=f"poutT{sfx}", bufs=1))
        oh_t = outT_pool.tile([P, NE, T], F8, tag="oh", name=f"oh{sfx}")
        ol_t = outT_pool.tile([P, NE, T], F8, tag="ol", name=f"ol{sfx}")

        # AV (bf16, pre-scaled by OS): split PSUM into fp8 high + residual
        for eb in range(NE):
            psA = [psum.tile([P, QCH], F32, tag="mm", name=f"ps_o{eb}_{qc}{sfx}")
                   for qc in range(NQC)]
            for kt in range(NT):
                for qc in range(NQC):
                    if kt >= (qc + 1) * 4:
                        continue
                    nc.tensor.matmul(
                        psA[qc][:],
                        v_t[kt][:, eb * P:(eb + 1) * P],
                        wT_t[kt][:, qc * QCH:(qc + 1) * QCH],
                        start=(kt == 0), stop=(kt == (qc + 1) * 4 - 1),
                    )
            for qc in range(NQC):
                ohs = oh_t[:, eb, qc * QCH:(qc + 1) * QCH]
                ols = ol_t[:, eb, qc * QCH:(qc + 1) * QCH]
                nc.scalar.copy(ohs, psA[qc][:])
                nc.vector.tensor_sub(ols, psA[qc][:], ohs)

        p2.close()

        p4 = ExitStack()
        ro_pool = p4.enter_context(tc.tile_pool(name=f"pro{sfx}", bufs=2))
        ostg_pool = p4.enter_context(tc.tile_pool(name=f"postg{sfx}", bufs=4))

        VGRP = 4
        for vg in range(NVC // VGRP):
            wh_t = ro_pool.tile([P, NE, VGRP, VCH], F8, tag="wh", name=f"wh{vg}{sfx}")
            wl_t = ro_pool.tile([P, NE, VGRP, VCH], F8, tag="wl", name=f"wl{vg}{sfx}")
            for e in range(NE):
                nc.sync.dma_start(wh_t[:, e, :, :],
                                  wro8h_d[:, e, vg * VGRP:(vg + 1) * VGRP, :])
                nc.sync.dma_start(wl_t[:, e, :, :],
                                  wro8l_d[:, e, vg * VGRP:(vg + 1) * VGRP, :])
            for tb in range(NT):
                psA = [psum.tile([P, VCH], F32, tag="mm", name=f"ps_r{vg}_{tb}_{vq}{sfx}")
                       for vq in range(VGRP)]
                tbs = slice(tb * P, (tb + 1) * P)
                # o_h stationary: main + W-residual terms share the weights
                for ep in range(NEP):
                    ohp = oh_t[:, 2 * ep:2 * ep + 2, tbs]
                    for vq in range(VGRP):
                        nc.tensor.matmul(
                            psA[vq][:], ohp, wh_t[:, 2 * ep:2 * ep + 2, vq, :],
                            start=(ep == 0), stop=False, perf_mode=DR)
                    for vq in range(VGRP):
                        nc.tensor.matmul(
                            psA[vq][:], ohp, wl_t[:, 2 * ep:2 * ep + 2, vq, :],
                            start=False, stop=False, perf_mode=DR)
                # o_l stationary: o-residual term
                for ep in range(NEP):
                    olp = ol_t[:, 2 * ep:2 * ep + 2, tbs]
                    for vq in range(VGRP):
                        nc.tensor.matmul(
                            psA[vq][:], olp, wh_t[:, 2 * ep:2 * ep + 2, vq, :],
                            start=False, stop=(ep == NEP - 1), perf_mode=DR)
                for vq in range(VGRP):
                    vc = vg * VGRP + vq
                    stg = ostg_pool.tile([P, VCH], BF16, tag="ostg",
                                         name=f"ostg{vc}_{tb}{sfx}")
                    if vq % 2 == 0:
                        nc.vector.tensor_scalar_mul(stg[:], psA[vq][:], S_RO)
                    else:
                        nc.scalar.activation(stg[:], psA[vq][:], Copy,
                                             bias=0.0, scale=S_RO)
                    nc.sync.dma_start(out_d[tb, :, vc * VCH:(vc + 1) * VCH], stg[:])

        p4.close()
        p3.close()


_EMITTERS = {"cc": _emit_body_cc, "rep": _emit_body_rep, "rep2": _emit_body_rep2,
             "rep3": _emit_body_rep3, "rep4": _emit_body_rep4}


def _build_program(mode=MODE, reps=1):
    nc = bacc.Bacc("TRN2", target_bir_lowering=False, debug=False, num_devices=8)

    if mode in ("rep3", "rep4"):
        xT_d = nc.dram_tensor("xT", [NE, P, T], BF16, kind="ExternalInput").ap()
        xT8_d = nc.dram_tensor("xT8", [P, NE, T], F8, kind="ExternalInput").ap()
        wk8_d = nc.dram_tensor("wk8", [P, NE, E], F8, kind="ExternalInput").ap()
        wq8_d = nc.dram_tensor("wq8", [P, NE, E], F8, kind="ExternalInput").ap()
        wvp_d = nc.dram_tensor("wvp", [P, NE, E], BF16, kind="ExternalInput").ap()
        mask_d = nc.dram_tensor("mask4", [P, NQC, QCH], F32, kind="ExternalInput").ap()
        out_d = nc.dram_tensor("logits", [NT, P, VS], BF16, kind="ExternalOutput").ap()
        if mode == "rep3":
            wro_d = nc.dram_tensor("wro", [NE, P, VS], BF16, kind="ExternalInput").ap()
            aps = (xT_d, xT8_d, wk8_d, wq8_d, wvp_d, wro_d, mask_d, out_d)
        else:
            wro8h_d = nc.dram_tensor("wro8h", [P, NE, NVC, VCH], F8,
                                     kind="ExternalInput").ap()
            wro8l_d = nc.dram_tensor("wro8l", [P, NE, NVC, VCH], F8,
                                     kind="ExternalInput").ap()
            aps = (xT_d, xT8_d, wk8_d, wq8_d, wvp_d, wro8h_d, wro8l_d,
                   mask_d, out_d)
        emit = _EMITTERS[mode]
        with tile.TileContext(nc) as tc:
            for r in range(reps):
                emit(tc, nc, aps, f"_r{r}" if reps > 1 else "")
        nc.compile()
        return nc

    xT_d = nc.dram_tensor("xT", [NE, P, T], BF16, kind="ExternalInput").ap()
    xTk_d = (nc.dram_tensor("xTk", [NE, P, TK], BF16, kind="ExternalInput").ap()
             if mode == "cc" else None)
    wk_d = nc.dram_tensor("wk", [NE, P, E], BF16, kind="ExternalInput").ap()
    wq_d = nc.dram_tensor("wq", [NE, P, E], BF16, kind="ExternalInput").ap()
    wv_d = nc.dram_tensor("wv", [NE, P, E], BF16, kind="ExternalInput").ap()
    wro_d = nc.dram_tensor("wro", [NE, P, VS], BF16, kind="ExternalInput").ap()
    if mode == "cc":
        mask_d = nc.dram_tensor("mask", [P, QCH], F32, kind="ExternalInput").ap()
    else:
        mask_d = nc.dram_tensor("mask4", [P, NQC, QCH], F32, kind="ExternalInput").ap()
    out_d = nc.dram_tensor("logits", [NT, P, VS], F32, kind="ExternalOutput").ap()

    aps = (xT_d, xTk_d, wk_d, wq_d, wv_d, wro_d, mask_d, out_d)
    emit = _EMITTERS[mode]

    with tile.TileContext(nc) as tc:
        for r in range(reps):
            emit(tc, nc, aps, f"_r{r}" if reps > 1 else "")

    nc.compile()
    return nc


def _get_nc():
    if "nc" not in _CACHE:
        _CACHE["nc"] = _build_program()
    return _CACHE["nc"]


def _make_in_maps_rep3(X, emb_table, pos_table, Wk, Wq, Wv, Wro):
    bf = ml_dtypes.bfloat16
    f8 = ml_dtypes.float8_e4m3
    X = np.asarray(X)
    emb_table = np.asarray(emb_table, np.float32)
    pos_table = np.asarray(pos_table, np.float32)

    x = emb_table[X] + pos_table[None, :, :]            # [B, T, E] f32

    wk8 = np.ascontiguousarray(
        np.asarray(Wk, np.float32).reshape(NE, P, E).transpose(1, 0, 2) * WS
    ).astype(f8)
    wq8 = np.ascontiguousarray(
        np.asarray(Wq, np.float32).reshape(NE, P, E).transpose(1, 0, 2) * WS
    ).astype(f8)
    wvp = np.ascontiguousarray(
        np.asarray(Wv, np.float32).reshape(NE, P, E).transpose(1, 0, 2)
    ).astype(bf)

    Wro = np.asarray(Wro, np.float32)
    wro_s, wro_h, wro_l = [], [], []
    for s in range(VSPLIT):
        sl = Wro[:, s * VS:(s + 1) * VS].reshape(NE, P, VS)
        if MODE == "rep4":
            scaled = np.ascontiguousarray(
                sl.transpose(1, 0, 2) * ROW_S)              # [P, NE, VS]
            h = scaled.astype(f8)
            l = (scaled - h.astype(np.float32)).astype(f8)
            wro_h.append(np.ascontiguousarray(h.reshape(P, NE, NVC, VCH)))
            wro_l.append(np.ascontiguousarray(l.reshape(P, NE, NVC, VCH)))
        else:
            wro_s.append(np.ascontiguousarray(sl).astype(bf))

    xT_b, xT8_b = [], []
    for b in range(B):
        xt = np.ascontiguousarray(x[b].T)                       # [E, T] f32
        xT_b.append(xt.reshape(NE, P, T).astype(bf))
        xT8_b.append(np.ascontiguousarray(
            xt.reshape(NE, P, T).transpose(1, 0, 2) * XS).astype(f8))

    p_idx = np.arange(P)[:, None]
    c_idx = np.arange(QCH)[None, :]
    masks = [
        np.where(c_idx < dv * P + p_idx, MASK_R3, 0.0).astype(np.float32)
        for dv in range(VSPLIT)
    ]
    mask4 = np.stack(masks, axis=1)                             # [P, NQC, QCH]

    in_maps = []
    for c in range(8):
        b, dv = divmod(c, VSPLIT)
        m = {
            "xT": xT_b[b],
            "xT8": xT8_b[b],
            "wk8": wk8, "wq8": wq8, "wvp": wvp,
            "mask4": mask4,
        }
        if MODE == "rep4":
            m["wro8h"] = wro_h[dv]
            m["wro8l"] = wro_l[dv]
        else:
            m["wro"] = wro_s[dv]
        in_maps.append(m)
    return in_maps


def _make_in_maps(X, emb_table, pos_table, Wk, Wq, Wv, Wro):
    if MODE in ("rep3", "rep4"):
        return _make_in_maps_rep3(X, emb_table, pos_table, Wk, Wq, Wv, Wro)
    bf = ml_dtypes.bfloat16
    X = np.asarray(X)
    emb_table = np.asarray(emb_table, np.float32)
    pos_table = np.asarray(pos_table, np.float32)

    # host-side embedding gather + positional add (0.03% of model FLOPs)
    x = emb_table[X] + pos_table[None, :, :]            # [B, T, E] f32

    wk = np.ascontiguousarray(np.asarray(Wk, np.float32).reshape(NE, P, E)).astype(bf)
    wq = np.ascontiguousarray(np.asarray(Wq, np.float32).reshape(NE, P, E)).astype(bf)
    wv = np.ascontiguousarray(np.asarray(Wv, np.float32).reshape(NE, P, E)).astype(bf)

    Wro = np.asarray(Wro, np.float32)
    wro_s = []
    for s in range(VSPLIT):
        sl = Wro[:, s * VS:(s + 1) * VS].reshape(NE, P, VS)
        wro_s.append(np.ascontiguousarray(sl).astype(bf))

    xT_b, xTk_b = [], []
    for b in range(B):
        xt = np.ascontiguousarray(x[b].T)                       # [E, T] f32
        xT_b.append(xt.reshape(NE, P, T).astype(bf))
        per_dv = []
        for dv in range(VSPLIT):
            cols = np.concatenate(
                [xt[:, (dv + 4 * j) * P:(dv + 4 * j + 1) * P] for j in range(KL)],
                axis=1,
            )                                                   # [E, TK]
            per_dv.append(np.ascontiguousarray(cols).reshape(NE, P, TK).astype(bf))
        xTk_b.append(per_dv)

    # staircase masks: masked iff col < dv*128 + p (diag chunk of k-tile dv+4j)
    p_idx = np.arange(P)[:, None]
    c_idx = np.arange(QCH)[None, :]
    masks = [
        np.where(c_idx < dv * P + p_idx, MASK_VAL, 0.0).astype(np.float32)
        for dv in range(VSPLIT)
    ]
    mask4 = np.stack(masks, axis=1)                             # [P, NQC, QCH]

    in_maps = []
    for c in range(8):
        b, dv = divmod(c, VSPLIT)
        in_maps.append({
            "xT": xT_b[b],
            "xTk": xTk_b[b][dv],
            "wk": wk, "wq": wq, "wv": wv,
            "wro": wro_s[dv],
            "mask": masks[dv],
            "mask4": mask4,
        })
    return in_maps


def run_on_device(in_maps, trace=False, **kw):
    nc = _get_nc()
    return run_bass_kernel_spmd(nc, in_maps, core_ids=list(range(8)), trace=trace, **kw)


def _unshard(results):
    logits = np.empty((B, T, VOC), np.float32)
    for c in range(8):
        b, s = divmod(c, VSPLIT)
        logits[b, :, s * VS:(s + 1) * VS] = (
            results[c]["logits"].reshape(T, VS).astype(np.float32))
    return logits


def kernel(X, emb_table, pos_table, Wk, Wq, Wv, Wro, bro):
    in_maps = _make_in_maps(X, emb_table, pos_table, Wk, Wq, Wv, Wro)
    _CACHE["in_maps"] = in_maps

    res = run_on_device(in_maps, trace=False)
    _CACHE["last_results"] = res

    logits = _unshard(res.results)

    bro = np.asarray(bro, np.float32)
    if np.any(bro):
        logits += bro
    return logits

